# revision 1
# baseline (speedup 1.0000x reference)
"""Trainium2 Bass kernel for nn_CrossAttention_17033840296537.

Full-input contract: kernel(**inputs) takes the unsharded tensors as in
reference.setup_inputs() and returns the full [8, 2048, 512] output.

Sharding: data-parallel over batch B=8 across the 8 NeuronCores (one
batch element per core). Weights are replicated.

Per-core algorithm (all matmuls in float32r, N=512 free dims):
  prologue (on-chip PE transposes, scratch in DRAM):
    qk_w^T  [512c, 512hd]   (SBUF, used for projections)
    fc_w^T  [4096hd, 512o]  -> DRAM scratch
    v_w^T   [512c, 4096hd]  -> DRAM scratch
    q^T, k^T (chunked)      -> projected immediately:
    qh^T = qk_w @ q^T [512hd, 2048q] -> DRAM scratch (same for kh^T)
  main loop over heads h, query chunks j (512 wide):
    scores^T[s, q] = kh^T[h].T-slices @ qh^T[h]   (K=64, direct in [s,q]
        layout so softmax needs no transpose)
    P^T = exp(scores^T * 0.125 + (-1e4 * mask)[s])  one ACT op, fused
        temperature + mask; no max-subtraction needed (|scores/8| <~ 8).
    T1[c, q]   = sum_s v[s, c] P^T[s, q]          (lhsT = v as stored!)
    r[q]       = sum_s P^T[s, q]                  (ones-vector matmul)
    out_h^T    = wv[h]^T-slices @ T1  then * (1/r) broadcast
    fc partial = out_h^T-slices @ fc_w^T[h]  accumulated over h in SBUF
  epilogue: += idt, LayerNorm, -> out.
"""

import numpy as np

import concourse.bass as bass
import concourse.tile as tile
from concourse import mybir
from concourse.bass import ds
from concourse.masks import make_identity

F32 = mybir.dt.float32
FR = mybir.dt.float32r
I32 = mybir.dt.int32
AF = mybir.ActivationFunctionType

B = 8
NQ = NS = 2048
DIM = 512          # input channel dim (DIM_K == DIM_V == 512)
N_HEAD = 8
D_K = 64
D_V = 512
HD = N_HEAD * D_V  # 4096 concat dim
P = 128


def fr(ap):
    return ap.bitcast(FR)


def _emit(tc: tile.TileContext, io: dict):
    nc = tc.nc
    q, k, v, mask, idt = io["q"], io["k"], io["v"], io["mask"], io["idt"]
    qk_w, v_w, fc_w = io["qk_w"], io["v_w"], io["fc_w"]
    fc_b, ln_g, ln_b = io["fc_b"], io["ln_g"], io["ln_b"]
    out = io["out"]
    qhT_d, khT_d = io["qhT_d"], io["khT_d"]

    cpool_cm = tc.tile_pool(name="cpool", bufs=1)
    vpool_cm = tc.tile_pool(name="vpool", bufs=1)
    cpool = cpool_cm.__enter__()
    vpool = vpool_cm.__enter__()

    # ---- constants ----
    ident = cpool.tile([P, P], F32)
    make_identity(nc, ident)
    ones_f = cpool.tile([P, P], F32)
    nc.vector.memset(ones_f, 1.0)
    ones32 = cpool.tile([P, 32], FR)
    nc.vector.tensor_copy(out=ones32, in_=ones_f[:, 0:32])
    ones_row = cpool.tile([1, P], FR)
    nc.vector.tensor_copy(out=ones_row, in_=ones_f[0:1, :])
    eps_t = cpool.tile([P, 1], F32)
    nc.vector.memset(eps_t, 1e-5)

    mask_i = cpool.tile([P, 16], I32)
    nc.sync.dma_start(out=mask_i, in_=mask.rearrange("(a p) -> p a", p=P))
    mask_b = cpool.tile([P, 16], F32)
    nc.vector.tensor_copy(out=mask_b, in_=mask_i)  # int32 -> f32 cast
    nc.scalar.mul(mask_b, mask_b, -10000.0)

    def bcast_row(name, src):  # [512] dram -> [128, 512] sbuf (rows identical)
        bc = cpool.tile([P, D_V], F32, name=name + "_bc")
        src_b = bass.AP(tensor=src.tensor, offset=src.offset,
                        ap=[[0, P]] + list(src.ap))
        nc.sync.dma_start(out=bc, in_=src_b)
        return bc

    fcb_bc = bcast_row("fcb", fc_b)
    lng_bc = bcast_row("lng", ln_g)
    lnb_bc = bcast_row("lnb", ln_b)

    # ---- resident: v tiles and fc accumulator ----
    vt = []
    for sb in range(16):
        vstg = vpool.tile([P, DIM], F32, name=f"vstg{sb}", tag="vstg", bufs=3)
        nc.gpsimd.dma_start(out=vstg, in_=v[ds(sb * P, P), :])
        vts = vpool.tile([P, DIM], FR, name=f"v{sb}", tag=f"v{sb}")
        nc.vector.tensor_copy(out=vts, in_=vstg)
        vt.append(vts)
    facc = [vpool.tile([P, D_V], F32, name=f"facc{i}", tag=f"facc{i}")
            for i in range(16)]

    # ================= prologue =================
    with (
        tc.tile_pool(name="ld", bufs=2) as ld,
        tc.tile_pool(name="ppsum", bufs=1, space="PSUM") as ppsum,
        tc.tile_pool(name="wpool", bufs=1) as wpool,
    ):
        # qk_w^T [c, hd] stays in SBUF for the projections below
        qkwT = [wpool.tile([P, 512], FR, name=f"qkwT{cb}", tag=f"qkwT{cb}")
                for cb in range(4)]
        wrows = []
        for rb in range(4):
            wrow = ld.tile([P, 512], F32, tag="wrow", bufs=4)
            nc.sync.dma_start(out=wrow, in_=qk_w[ds(rb * P, P), :])
            wrows.append(wrow)
        for cb in range(4):
            tp4 = ppsum.tile([P, 512], F32, tag="tp4", bufs=3)
            for rb in range(4):
                nc.tensor.transpose(tp4[:, ds(rb * P, P)],
                                    wrows[rb][:, ds(cb * P, P)], ident)
            nc.vector.tensor_copy(out=qkwT[cb], in_=tp4)

        # q, k: transpose chunks + project through qk_w^T -> qh^T/kh^T scratch
        for src, dst in ((q, qhT_d), (k, khT_d)):
            for j2 in range(4):  # chunks of 512 sequence rows
                qts = []
                for qb in range(4):
                    qt = ld.tile([P, DIM], F32, tag="qld", bufs=8)
                    nc.gpsimd.dma_start(out=qt, in_=src[ds((j2 * 4 + qb) * P, P), :])
                    qts.append(qt)
                qTc = [ld.tile([P, 512], FR, name=f"qTc{cb}", tag=f"qTc{cb}")
                       for cb in range(4)]
                for cb in range(4):
                    tp4 = ppsum.tile([P, 512], F32, tag="tp4", bufs=3)
                    for qb in range(4):
                        nc.tensor.transpose(tp4[:, ds(qb * P, P)],
                                            qts[qb][:, ds(cb * P, P)], ident)
                    nc.vector.tensor_copy(out=qTc[cb], in_=tp4)
                for mb in range(4):
                    pr = ppsum.tile([P, 512], F32, tag="pr", bufs=2)
                    for cb in range(4):
                        nc.tensor.matmul(pr, lhsT=qkwT[cb][:, ds(mb * P, P)],
                                         rhs=qTc[cb],
                                         start=(cb == 0), stop=(cb == 3))
                    prs = ld.tile([P, 512], FR, tag="prs", bufs=2)
                    nc.vector.tensor_copy(out=prs, in_=pr)
                    nc.sync.dma_start(out=dst[ds(mb * P, P), ds(j2 * 512, 512)],
                                      in_=prs)

    # ================= main =================
    with (
        tc.tile_pool(name="mpsum", bufs=1, space="PSUM") as mpsum,
        tc.tile_pool(name="hpool", bufs=1) as hpool,
        tc.tile_pool(name="spool", bufs=2) as spool,
    ):
        for h in range(N_HEAD):
            # qh^T / kh^T for this head, duplicated into both partition
            # halves so paired score matmuls can row-tile the PE array.
            qh2 = hpool.tile([P, NQ], FR, tag="qh", bufs=2)
            nc.gpsimd.dma_start(out=qh2[0:D_K, :], in_=qhT_d[ds(h * D_K, D_K), :])
            nc.gpsimd.dma_start(out=qh2[D_K:P, :], in_=qhT_d[ds(h * D_K, D_K), :])
            kh2 = hpool.tile([P, NS], FR, tag="kh", bufs=2)
            nc.gpsimd.dma_start(out=kh2[0:D_K, :], in_=khT_d[ds(h * D_K, D_K), :])
            nc.gpsimd.dma_start(out=kh2[D_K:P, :], in_=khT_d[ds(h * D_K, D_K), :])

            # transpose this head's slices of v_w and fc_w on the fly
            wvT = [hpool.tile([P, 512], FR, name=f"wvT{cb}", tag=f"wv{cb}")
                   for cb in range(4)]
            vwr = []
            for i in range(4):
                vw_raw = hpool.tile([P, 512], F32, tag=f"raw{i}")
                nc.gpsimd.dma_start(out=vw_raw, in_=v_w[ds((h * 4 + i) * P, P), :])
                vwr.append(vw_raw)
            for cb in range(4):
                tp4 = mpsum.tile([P, 512], F32, tag="sc", bufs=2)
                for i in range(4):
                    nc.tensor.transpose(tp4[:, ds(i * P, P)],
                                        vwr[i][:, ds(cb * P, P)], ident)
                nc.vector.tensor_copy(out=wvT[cb], in_=tp4)
            fwT = [hpool.tile([P, 512], FR, name=f"fwT{db}", tag=f"fw{db}")
                   for db in range(4)]
            fwr = []
            for rb in range(4):
                fw_raw = hpool.tile([P, 512], F32, tag=f"raw{rb}")
                nc.gpsimd.dma_start(out=fw_raw,
                                  in_=fc_w[ds(rb * P, P), ds(h * 512, 512)])
                fwr.append(fw_raw)
            for db in range(4):
                tp4 = mpsum.tile([P, 512], F32, tag="sc", bufs=2)
                for rb in range(4):
                    nc.tensor.transpose(tp4[:, ds(rb * P, P)],
                                        fwr[rb][:, ds(db * P, P)], ident)
                nc.vector.tensor_copy(out=fwT[db], in_=tp4)

            for j in range(4):  # query chunks of 512
                t1 = mpsum.tile([P, 4 * 512], F32, tag="t1", bufs=1)
                r2a = mpsum.tile([32, 512], F32, tag="oo", bufs=2)

                def emit_sc_pair(si):
                    # paired score matmuls row-tiled into PE halves
                    sba, sbb = 2 * si, 2 * si + 1
                    sc_a = mpsum.tile([P, 512], F32, tag="sc", bufs=2,
                                      name=f"sc_a{si}")
                    nc.tensor.matmul(sc_a, lhsT=kh2[0:D_K, ds(sba * P, P)],
                                     rhs=qh2[0:D_K, ds(j * 512, 512)],
                                     start=True, stop=True,
                                     tile_position=(0, 0))
                    sc_b = mpsum.tile([P, 512], F32, tag="sc", bufs=2,
                                      name=f"sc_b{si}")
                    nc.tensor.matmul(sc_b, lhsT=kh2[D_K:P, ds(sbb * P, P)],
                                     rhs=qh2[D_K:P, ds(j * 512, 512)],
                                     start=True, stop=True,
                                     tile_position=(D_K, 0))
                    return sc_a, sc_b

                def emit_half(sb, sc, rrow):
                    pt = spool.tile([P, 512], FR, tag="pt", bufs=6)
                    nc.scalar.activation(pt, sc, AF.Exp,
                                         bias=mask_b[:, ds(sb, 1)],
                                         scale=0.125)
                    for cb in range(4):
                        nc.tensor.matmul(t1[:, ds(cb * 512, 512)],
                                         lhsT=vt[sb][:, ds(cb * P, P)],
                                         rhs=pt,
                                         start=(sb == 0), stop=(sb == 15))
                    # softmax denominator partials (one accumulation group)
                    nc.tensor.matmul(r2a[0:32, :], lhsT=ones32, rhs=pt,
                                     start=(sb == 0), stop=(sb == 15))

                pair = emit_sc_pair(0)
                for si in range(8):
                    sc_a, sc_b = pair
                    emit_half(2 * si, sc_a, 0)
                    if si < 7:
                        pair = emit_sc_pair(si + 1)
                    emit_half(2 * si + 1, sc_b, 32)

                rs = spool.tile([1, 512], FR, tag="rs")
                with nc.allow_low_precision(reason="f32r rounding of 1/r"):
                    nc.vector.reciprocal(rs, r2a[0:1, :])
                # broadcast 1/r across partitions via K=1 ones-matmul, then
                # move to SBUF right away so the PSUM slot frees quickly
                rb_t = mpsum.tile([P, 512], F32, tag="oo", bufs=2)
                nc.tensor.matmul(rb_t, lhsT=ones_row, rhs=rs,
                                 start=True, stop=True)
                rbs = spool.tile([P, 512], F32, tag="rbs", bufs=2)
                nc.vector.tensor_copy(out=rbs, in_=rb_t)
                # t1 PSUM -> SBUF on the scalar engine (idle here), chunked so
                # stage-2 matmuls can start on chunk 0 early
                t1s = spool.tile([P, 4 * 512], FR, tag="t1s", bufs=2)
                for cb in range(4):
                    nc.scalar.copy(out=t1s[:, ds(cb * 512, 512)],
                                   in_=t1[:, ds(cb * 512, 512)])

                oTs = []
                for db in range(4):
                    oo = mpsum.tile([P, 512], F32, tag="oo", bufs=2)
                    for cb in range(4):
                        nc.tensor.matmul(oo, lhsT=wvT[cb][:, ds(db * P, P)],
                                         rhs=t1s[:, ds(cb * 512, 512)],
                                         start=(cb == 0), stop=(cb == 3))
                    oT = spool.tile([P, 512], FR, name=f"oT{db}", tag=f"oT{db}")
                    nc.vector.tensor_mul(oT, oo, rbs)
                    oTs.append(oT)

                for qb in range(4):
                    fp = mpsum.tile([P, 512], F32, tag="oo", bufs=2)
                    for db in range(4):
                        nc.tensor.matmul(fp, lhsT=oTs[db][:, ds(qb * P, P)],
                                         rhs=fwT[db],
                                         start=(db == 0), stop=(db == 3))
                    i16 = j * 4 + qb
                    if h == 0:
                        nc.vector.tensor_add(facc[i16], fp, fcb_bc)
                    else:
                        nc.vector.tensor_add(facc[i16], fp, facc[i16])

        # ---- epilogue: residual + LayerNorm ----
        for i16 in range(16):
            it = spool.tile([P, D_V], F32, tag="it")
            nc.gpsimd.dma_start(out=it, in_=idt[ds(i16 * P, P), :])
            xt = spool.tile([P, D_V], F32, tag="xt")
            nc.vector.tensor_add(xt, facc[i16], it)
            st = spool.tile([P, 6], F32, tag="st")
            nc.vector.bn_stats(out=st, in_=xt)
            mv = spool.tile([P, 2], F32, tag="mv")
            nc.vector.bn_aggr(out=mv, in_=st)
            sd = spool.tile([P, 1], F32, tag="sd")
            nc.scalar.activation(sd, mv[:, 1:2], AF.Sqrt, bias=eps_t)
            rstd = spool.tile([P, 1], F32, tag="rstd")
            nc.vector.reciprocal(rstd, sd)
            nc.vector.tensor_scalar(out=xt, in0=xt, scalar1=mv[:, 0:1],
                                    scalar2=rstd,
                                    op0=mybir.AluOpType.subtract,
                                    op1=mybir.AluOpType.mult)
            nc.vector.tensor_mul(xt, xt, lng_bc)
            nc.vector.tensor_add(xt, xt, lnb_bc)
            nc.sync.dma_start(out=out[ds(i16 * P, P), :], in_=xt)

    vpool_cm.__exit__(None, None, None)
    cpool_cm.__exit__(None, None, None)


def build_nc():
    from concourse import bacc
    nc = bacc.Bacc("TRN2", target_bir_lowering=False, debug=False)
    io = {}
    io["q"] = nc.dram_tensor("q", [NQ, DIM], F32, kind="ExternalInput").ap()
    io["k"] = nc.dram_tensor("k", [NS, DIM], F32, kind="ExternalInput").ap()
    io["v"] = nc.dram_tensor("v", [NS, DIM], F32, kind="ExternalInput").ap()
    io["mask"] = nc.dram_tensor("mask", [NS], I32, kind="ExternalInput").ap()
    io["idt"] = nc.dram_tensor("idt", [NQ, D_V], F32, kind="ExternalInput").ap()
    io["qk_w"] = nc.dram_tensor("qk_w", [512, DIM], F32, kind="ExternalInput").ap()
    io["v_w"] = nc.dram_tensor("v_w", [HD, DIM], F32, kind="ExternalInput").ap()
    io["fc_w"] = nc.dram_tensor("fc_w", [D_V, HD], F32, kind="ExternalInput").ap()
    io["fc_b"] = nc.dram_tensor("fc_b", [D_V], F32, kind="ExternalInput").ap()
    io["ln_g"] = nc.dram_tensor("ln_g", [D_V], F32, kind="ExternalInput").ap()
    io["ln_b"] = nc.dram_tensor("ln_b", [D_V], F32, kind="ExternalInput").ap()
    io["out"] = nc.dram_tensor("out", [NQ, D_V], F32, kind="ExternalOutput").ap()
    io["qhT_d"] = nc.dram_tensor("qhT_d", [512, NQ], FR).ap()
    io["khT_d"] = nc.dram_tensor("khT_d", [512, NS], FR).ap()

    with tile.TileContext(nc) as tc:
        _emit(tc, io)
    nc.compile()
    return nc


_NC = None


def get_nc():
    global _NC
    if _NC is None:
        _NC = build_nc()
    return _NC


def make_in_maps(q, k, v, s_valid_mask, idt, qk_w, v_w, fc_w, fc_b, ln_g, ln_b):
    in_maps = []
    for b in range(B):
        in_maps.append({
            "q": np.ascontiguousarray(q[b], dtype=np.float32),
            "k": np.ascontiguousarray(k[b], dtype=np.float32),
            "v": np.ascontiguousarray(v[b], dtype=np.float32),
            "mask": np.ascontiguousarray(s_valid_mask[b], dtype=np.int32),
            "idt": np.ascontiguousarray(idt[b], dtype=np.float32),
            "qk_w": np.ascontiguousarray(qk_w, dtype=np.float32),
            "v_w": np.ascontiguousarray(v_w, dtype=np.float32),
            "fc_w": np.ascontiguousarray(fc_w, dtype=np.float32),
            "fc_b": np.ascontiguousarray(fc_b, dtype=np.float32),
            "ln_g": np.ascontiguousarray(ln_g, dtype=np.float32),
            "ln_b": np.ascontiguousarray(ln_b, dtype=np.float32),
        })
    return in_maps


def kernel(q, k, v, s_valid_mask, idt, qk_w, v_w, fc_w, fc_b, ln_g, ln_b,
           **run_kwargs):
    from concourse.bass_utils import run_bass_kernel_spmd

    nc = get_nc()
    in_maps = make_in_maps(q, k, v, s_valid_mask, idt,
                           qk_w, v_w, fc_w, fc_b, ln_g, ln_b)
    res = run_bass_kernel_spmd(nc, in_maps, core_ids=list(range(B)),
                               **run_kwargs)
    out = np.stack([res.results[b]["out"] for b in range(B)], axis=0)
    kernel.last_results = res
    return out.astype(np.float32)



# revision 14
# speedup vs baseline: 1.2002x; 1.2002x over previous
"""Trainium2 Bass kernel for nn_CrossAttention_17033840296537.

Full-input contract: kernel(**inputs) takes the unsharded tensors as in
reference.setup_inputs() and returns the full [8, 2048, 512] output.

Sharding: data-parallel over batch B=8 across the 8 NeuronCores (one
batch element per core). Weights are replicated.

Per-core design (bf16 matmul operands, f32 PSUM accumulation):
  prologue (everything SBUF-resident, no DRAM scratch):
    qk_w^T -> qkwT (bf16)
    q^T, k^T via PE transposes, projected to qhT/khT [512hd, 2048] bf16
    v cast to vt bf16 (lhsT for attn@V as stored)
    M_h = wv_h^T @ fc_w[:,h]^T  [512c, 512o] bf16 per head -- merges the
        v-projection and the output fc into ONE matmul stage downstream.
  main loop, j (512-query chunk) outer, h inner:
    scores^T[s,q] = khT[h] slices^T @ qhT[h]   (K=64, tile_position by
        head parity -- no duplication needed)
    pt = exp(scores*0.125 + mask_bias)  bf16, UNNORMALIZED
    t1[c,q]  += vt[s,c]^T pt           (K=128, PSUM accum over 16 s-blk)
    r[q]     += ones^T pt              (softmax denominator, same pass)
    fc partial fp[q,o] = sum_cb t1s[cb]^T M_h[cb]  (16 matmuls)
    facc[q,o] = fp * (1/r)[q] + facc   (ONE fused DVE op; 1/r arrives as
        a per-partition column via 4 tiny K=1 matmuls + reciprocal,
        entirely off the PE critical path)
  fc/facc/epilogue work of iteration i is emitted interleaved into
  iteration i+1's score/t1 stream so the PE never drains.
  epilogue per j: += idt, LayerNorm (Rsqrt on scalar, batched), -> out.
"""

import numpy as np

import concourse.bass as bass
import concourse.tile as tile
from concourse import mybir
from concourse.bass import ds
from concourse.masks import make_identity

F32 = mybir.dt.float32
BF = mybir.dt.bfloat16
I32 = mybir.dt.int32
AF = mybir.ActivationFunctionType
ALU = mybir.AluOpType

B = 8
NQ = NS = 2048
DIM = 512          # input channel dim (DIM_K == DIM_V == 512)
N_HEAD = 8
D_K = 64
D_V = 512
HD = N_HEAD * D_V  # 4096 concat dim
P = 128


def _emit(tc: tile.TileContext, io: dict):
    nc = tc.nc
    q, k, v, mask, idt = io["q"], io["k"], io["v"], io["mask"], io["idt"]
    qk_w, v_w, fc_w = io["qk_w"], io["v_w"], io["fc_w"]
    fc_b, ln_g, ln_b = io["fc_b"], io["ln_g"], io["ln_b"]
    out = io["out"]

    cpool_cm = tc.tile_pool(name="cpool", bufs=1)
    rpool_cm = tc.tile_pool(name="rpool", bufs=1)
    cpool = cpool_cm.__enter__()
    rpool = rpool_cm.__enter__()

    # ---- constants ----
    ident = cpool.tile([P, P], F32, name="ident")
    make_identity(nc, ident)
    ones_f = cpool.tile([P, 1], F32, name="ones_f")
    nc.vector.memset(ones_f, 1.0)
    ones_col = cpool.tile([P, 1], BF, name="ones_col")
    nc.vector.tensor_copy(out=ones_col, in_=ones_f)
    one11 = cpool.tile([1, 1], F32, name="one11")
    nc.vector.memset(one11, 1.0)
    eps_t = cpool.tile([P, 1], F32, name="eps_t")
    nc.vector.memset(eps_t, 1e-5)

    mask_i = cpool.tile([P, 16], I32, name="mask_i")
    nc.sync.dma_start(out=mask_i, in_=mask.rearrange("(a p) -> p a", p=P))
    mask_b = cpool.tile([P, 16], F32, name="mask_b")
    nc.vector.tensor_copy(out=mask_b, in_=mask_i)  # int32 -> f32 cast
    nc.scalar.mul(mask_b, mask_b, -10000.0)

    def bcast_row(name, src):  # [512] dram -> [128, 512] sbuf (rows identical)
        bc = cpool.tile([P, D_V], F32, name=name + "_bc")
        src_b = bass.AP(tensor=src.tensor, offset=src.offset,
                        ap=[[0, P]] + list(src.ap))
        nc.sync.dma_start(out=bc, in_=src_b)
        return bc

    fcb_bc = bcast_row("fcb", fc_b)
    lng_bc = bcast_row("lng", ln_g)
    lnb_bc = bcast_row("lnb", ln_b)

    # ---- residents ----
    vt = [rpool.tile([P, DIM], BF, name=f"vt{sb}") for sb in range(16)]
    qhT = [rpool.tile([P, NQ], BF, name=f"qhT{mb}") for mb in range(4)]
    khT = [rpool.tile([P, NS], BF, name=f"khT{mb}") for mb in range(4)]
    Msb = [[rpool.tile([P, D_V], BF, name=f"M{h}_{cb}") for cb in range(4)]
           for h in range(N_HEAD)]
    facc = [rpool.tile([P, D_V], F32, name=f"facc{i}") for i in range(16)]

    # ================= prologue =================
    with (
        tc.tile_pool(name="pstage", bufs=1) as pstage,
        tc.tile_pool(name="ppsum", bufs=1, space="PSUM") as pp,
    ):
        # qk_w^T -> qkwT bf16
        wrows = []
        for rb in range(4):
            wrow = pstage.tile([P, 512], F32, name=f"wrow{rb}", tag="qld",
                               bufs=6)
            nc.sync.dma_start(out=wrow, in_=qk_w[ds(rb * P, P), :])
            wrows.append(wrow)
        qkwT = []
        for cb in range(4):
            tp = pp.tile([P, 512], F32, name=f"tpw{cb}", tag="tp", bufs=3)
            for rb in range(4):
                nc.tensor.transpose(tp[:, ds(rb * P, P)],
                                    wrows[rb][:, ds(cb * P, P)], ident)
            qw = pstage.tile([P, 512], BF, name=f"qkwT{cb}", tag=f"qkwT{cb}")
            nc.vector.tensor_copy(out=qw, in_=tp)
            qkwT.append(qw)

        # v: load + cast to bf16, both on the gpsimd queue
        for sb in range(16):
            st = pstage.tile([P, DIM], F32, name=f"vstg{sb}", tag="vstg",
                             bufs=3)
            nc.gpsimd.dma_start(out=st, in_=v[ds(sb * P, P), :])
            nc.gpsimd.tensor_copy(out=vt[sb], in_=st)

        # q, k: transpose chunks + project -> qhT/khT bf16 (SBUF resident)
        for src, dstT in ((q, qhT), (k, khT)):
            for j2 in range(4):  # chunks of 512 sequence rows
                stg = []
                for qb in range(4):
                    qt = pstage.tile([P, DIM], F32,
                                     name=f"qstg{j2}_{qb}", tag="qld", bufs=6)
                    nc.sync.dma_start(out=qt, in_=src[ds((j2 * 4 + qb) * P, P), :])
                    stg.append(qt)
                qTc = []
                for cb in range(4):
                    tp = pp.tile([P, 512], F32, name=f"tpq{j2}_{cb}",
                                 tag="tp", bufs=3)
                    for qb in range(4):
                        nc.tensor.transpose(tp[:, ds(qb * P, P)],
                                            stg[qb][:, ds(cb * P, P)], ident)
                    qc = pstage.tile([P, 512], BF, name=f"qTc{j2}_{cb}",
                                     tag="qTc", bufs=6)
                    nc.vector.tensor_copy(out=qc, in_=tp)
                    qTc.append(qc)
                for mb in range(4):
                    pr = pp.tile([P, 512], F32, name=f"pr{j2}_{mb}",
                                 tag="pr", bufs=2)
                    for cb in range(4):
                        nc.tensor.matmul(pr, lhsT=qkwT[cb][:, ds(mb * P, P)],
                                         rhs=qTc[cb],
                                         start=(cb == 0), stop=(cb == 3))
                    nc.scalar.copy(out=dstT[mb][:, ds(j2 * 512, 512)], in_=pr)

        # per-head merged projection M_h = wv_h^T @ fc_w[:, h]^T  [c, o]
        for h in range(N_HEAD):
            fstg = []
            for rb in range(4):
                ft = pstage.tile([P, 512], F32, name=f"fstg{h}_{rb}",
                                 tag="wstg", bufs=6)
                nc.gpsimd.dma_start(out=ft,
                                    in_=fc_w[ds(rb * P, P), ds(h * 512, 512)])
                fstg.append(ft)
            fwT = []
            for db in range(4):
                tp = pp.tile([P, 512], F32, name=f"tpf{h}_{db}",
                             tag="tp", bufs=3)
                for rb in range(4):
                    nc.tensor.transpose(tp[:, ds(rb * P, P)],
                                        fstg[rb][:, ds(db * P, P)], ident)
                fw = pstage.tile([P, 512], BF, name=f"fwT{h}_{db}",
                                 tag="fwT", bufs=6)
                nc.vector.tensor_copy(out=fw, in_=tp)
                fwT.append(fw)
            vwb = []
            for i in range(4):
                vw_raw = pstage.tile([P, 512], F32, name=f"vwstg{h}_{i}",
                                     tag="wstg", bufs=6)
                nc.gpsimd.dma_start(out=vw_raw,
                                    in_=v_w[ds((h * 4 + i) * P, P), :])
                vb = pstage.tile([P, 512], BF, name=f"vwb{h}_{i}",
                                 tag="vwb", bufs=6)
                nc.gpsimd.tensor_copy(out=vb, in_=vw_raw)
                vwb.append(vb)
            for cb in range(4):
                pr = pp.tile([P, 512], F32, name=f"prM{h}_{cb}",
                             tag="pr", bufs=2)
                for i in range(4):
                    nc.tensor.matmul(pr, lhsT=vwb[i][:, ds(cb * P, P)],
                                     rhs=fwT[i],
                                     start=(i == 0), stop=(i == 3))
                nc.scalar.copy(out=Msb[h][cb], in_=pr)

    # ================= main =================
    with (
        tc.tile_pool(name="ms", bufs=1) as ms,
        tc.tile_pool(name="mp", bufs=1, space="PSUM") as mp,
    ):
        iters = [(j, h) for j in range(4) for h in range(8)]
        idt_tiles = {}   # j -> [4 tiles]
        prev = None      # dict carrying previous iteration's state
        ep_pending = []  # j values whose epilogue is ready to emit

        def emit_idt_loads(j):
            tiles = []
            for qb in range(4):
                it = ms.tile([P, D_V], F32, name=f"idt{j}_{qb}", tag="idt",
                             bufs=4)
                nc.sync.dma_start(out=it, in_=idt[ds((j * 4 + qb) * P, P), :])
                tiles.append(it)
            idt_tiles[j] = tiles

        def emit_rcol(pv, idx):
            # previous iteration's softmax sums [1,512] -> per-partition
            # column [128,4] + reciprocal; rides the "fp" PSUM bank.
            rcolt = mp.tile([P, 512], F32, name=f"rcol{idx}", tag="fp",
                            bufs=1)
            for qb in range(4):
                nc.tensor.matmul(rcolt[:, ds(qb, 1)],
                                 lhsT=pv["rs"][0:1, ds(qb * P, P)],
                                 rhs=one11, start=True, stop=True)
            rinv = ms.tile([P, 4], F32, name=f"rinv{idx}", tag="rinv", bufs=2)
            nc.vector.reciprocal(rinv, rcolt[:, 0:4])
            pv["rinv"] = rinv

        def emit_fc_group(pv, qb, idx):
            fpt = mp.tile([P, 512], F32, name=f"fp{idx}_{qb}", tag="fp",
                          bufs=1)
            for cb in range(4):
                nc.tensor.matmul(fpt,
                                 lhsT=pv["t1s"][cb][:, ds(qb * P, P)],
                                 rhs=Msb[pv["h"]][cb],
                                 start=(cb == 0), stop=(cb == 3))
            i16 = pv["j"] * 4 + qb
            in1 = fcb_bc if pv["h"] == 0 else facc[i16]
            nc.vector.scalar_tensor_tensor(out=facc[i16], in0=fpt,
                                           scalar=pv["rinv"][:, ds(qb, 1)],
                                           in1=in1,
                                           op0=ALU.mult, op1=ALU.add)
            if pv["h"] == 7 and qb == 3:
                ep_pending.append(pv["j"])

        def emit_epilogue(j):
            # residual + LayerNorm for the 4 row-tiles of query-chunk j
            xts, mvs = [], []
            for qb in range(4):
                i16 = j * 4 + qb
                xt = ms.tile([P, D_V], F32, name=f"xt{i16}", tag="xt", bufs=4)
                nc.vector.tensor_add(xt, facc[i16], idt_tiles[j][qb])
                st = ms.tile([P, 6], F32, name=f"st{i16}", tag="st", bufs=4)
                nc.vector.bn_stats(out=st, in_=xt)
                mv = ms.tile([P, 2], F32, name=f"mv{i16}", tag="mv", bufs=4)
                nc.vector.bn_aggr(out=mv, in_=st)
                xts.append(xt)
                mvs.append(mv)
            sds = []
            for qb in range(4):  # batched so the scalar engine swaps its
                i16 = j * 4 + qb  # activation table Exp->Sqrt only once
                sd = ms.tile([P, 1], F32, name=f"sd{i16}", tag="sd", bufs=4)
                nc.scalar.activation(sd, mvs[qb][:, 1:2], AF.Sqrt,
                                     bias=eps_t)
                sds.append(sd)
            rstds = []
            for qb in range(4):
                i16 = j * 4 + qb
                rstd = ms.tile([P, 1], F32, name=f"rstd{i16}", tag="rstd",
                               bufs=4)
                nc.vector.reciprocal(rstd, sds[qb])
                rstds.append(rstd)
            for qb in range(4):
                i16 = j * 4 + qb
                xt = xts[qb]
                nc.vector.tensor_scalar(out=xt, in0=xt,
                                        scalar1=mvs[qb][:, 0:1],
                                        scalar2=rstds[qb],
                                        op0=ALU.subtract, op1=ALU.mult)
                nc.vector.tensor_mul(xt, xt, lng_bc)
                nc.vector.tensor_add(xt, xt, lnb_bc)
                nc.sync.dma_start(out=out[ds(i16 * P, P), :], in_=xt)

        for idx, (j, h) in enumerate(iters):
            par = h % 2
            tnum = h // 2
            po = par * D_K
            t1 = mp.tile([P, 4 * 512], F32, name=f"t1_{idx}", tag="t1",
                         bufs=1)
            r2a = mp.tile([1, 512], F32, name=f"r2a_{idx}", tag="r2a",
                          bufs=1)

            def emit_pair(si):
                tiles = []
                for sb in (2 * si, 2 * si + 1):
                    sct = mp.tile([P, 512], F32, name=f"sc{idx}_{sb}",
                                  tag="sc", bufs=2)
                    nc.tensor.matmul(sct,
                                     lhsT=khT[tnum][po:po + D_K,
                                                    ds(sb * P, P)],
                                     rhs=qhT[tnum][po:po + D_K,
                                                   ds(j * 512, 512)],
                                     start=True, stop=True,
                                     tile_position=(po, 0))
                    tiles.append(sct)
                return tiles

            def emit_half(sb, sct):
                ptt = ms.tile([P, 512], BF, name=f"pt{idx}_{sb}", tag="pt",
                              bufs=6)
                nc.scalar.activation(ptt, sct, AF.Exp,
                                     bias=mask_b[:, ds(sb, 1)], scale=0.125)
                for cb in range(4):
                    nc.tensor.matmul(t1[:, ds(cb * 512, 512)],
                                     lhsT=vt[sb][:, ds(cb * P, P)],
                                     rhs=ptt,
                                     start=(sb == 0), stop=(sb == 15))
                nc.tensor.matmul(r2a, lhsT=ones_col, rhs=ptt,
                                 start=(sb == 0), stop=(sb == 15))

            pair = emit_pair(0)
            if prev is not None:
                emit_rcol(prev, idx)
            for si in range(8):
                emit_half(2 * si, pair[0])
                if si < 7:
                    nxt = emit_pair(si + 1)
                if prev is not None and 1 <= si <= 4:
                    emit_fc_group(prev, si - 1, idx)
                if si == 5:
                    if h == 5:
                        emit_idt_loads(j)
                    if ep_pending and h >= 1:
                        emit_epilogue(ep_pending.pop(0))
                emit_half(2 * si + 1, pair[1])
                if si < 7:
                    pair = nxt

            # end of iteration: softmax sums out, t1 -> SBUF bf16 split
            # across scalar+vector (gpsimd cannot read PSUM) to minimize
            # the t1 PSUM free latency
            rs = ms.tile([1, 512], F32, name=f"rs{idx}", tag="rs", bufs=2)
            nc.vector.tensor_copy(out=rs, in_=r2a)
            t1s = [ms.tile([P, 512], BF, name=f"t1s{idx}_{cb}", tag="t1s",
                           bufs=8) for cb in range(4)]
            nc.scalar.copy(out=t1s[0], in_=t1[:, ds(0, 512)])
            nc.scalar.copy(out=t1s[1], in_=t1[:, ds(512, 512)])
            nc.vector.tensor_copy(out=t1s[2], in_=t1[:, ds(1024, 512)])
            nc.vector.tensor_copy(out=t1s[3], in_=t1[:, ds(1536, 512)])
            prev = {"j": j, "h": h, "t1s": t1s, "rs": rs, "rinv": None}

        # drain the last iteration + epilogue for j=3
        emit_rcol(prev, 32)
        for qb in range(4):
            emit_fc_group(prev, qb, 32)
        while ep_pending:
            emit_epilogue(ep_pending.pop(0))

    rpool_cm.__exit__(None, None, None)
    cpool_cm.__exit__(None, None, None)


def build_nc():
    from concourse import bacc
    nc = bacc.Bacc("TRN2", target_bir_lowering=False, debug=False)
    io = {}
    io["q"] = nc.dram_tensor("q", [NQ, DIM], F32, kind="ExternalInput").ap()
    io["k"] = nc.dram_tensor("k", [NS, DIM], F32, kind="ExternalInput").ap()
    io["v"] = nc.dram_tensor("v", [NS, DIM], F32, kind="ExternalInput").ap()
    io["mask"] = nc.dram_tensor("mask", [NS], I32, kind="ExternalInput").ap()
    io["idt"] = nc.dram_tensor("idt", [NQ, D_V], F32, kind="ExternalInput").ap()
    io["qk_w"] = nc.dram_tensor("qk_w", [512, DIM], F32, kind="ExternalInput").ap()
    io["v_w"] = nc.dram_tensor("v_w", [HD, DIM], F32, kind="ExternalInput").ap()
    io["fc_w"] = nc.dram_tensor("fc_w", [D_V, HD], F32, kind="ExternalInput").ap()
    io["fc_b"] = nc.dram_tensor("fc_b", [D_V], F32, kind="ExternalInput").ap()
    io["ln_g"] = nc.dram_tensor("ln_g", [D_V], F32, kind="ExternalInput").ap()
    io["ln_b"] = nc.dram_tensor("ln_b", [D_V], F32, kind="ExternalInput").ap()
    io["out"] = nc.dram_tensor("out", [NQ, D_V], F32, kind="ExternalOutput").ap()

    with tile.TileContext(nc) as tc:
        _emit(tc, io)
    nc.compile()
    return nc


_NC = None


def get_nc():
    global _NC
    if _NC is None:
        _NC = build_nc()
    return _NC


def make_in_maps(q, k, v, s_valid_mask, idt, qk_w, v_w, fc_w, fc_b, ln_g, ln_b):
    in_maps = []
    for b in range(B):
        in_maps.append({
            "q": np.ascontiguousarray(q[b], dtype=np.float32),
            "k": np.ascontiguousarray(k[b], dtype=np.float32),
            "v": np.ascontiguousarray(v[b], dtype=np.float32),
            "mask": np.ascontiguousarray(s_valid_mask[b], dtype=np.int32),
            "idt": np.ascontiguousarray(idt[b], dtype=np.float32),
            "qk_w": np.ascontiguousarray(qk_w, dtype=np.float32),
            "v_w": np.ascontiguousarray(v_w, dtype=np.float32),
            "fc_w": np.ascontiguousarray(fc_w, dtype=np.float32),
            "fc_b": np.ascontiguousarray(fc_b, dtype=np.float32),
            "ln_g": np.ascontiguousarray(ln_g, dtype=np.float32),
            "ln_b": np.ascontiguousarray(ln_b, dtype=np.float32),
        })
    return in_maps


def kernel(q, k, v, s_valid_mask, idt, qk_w, v_w, fc_w, fc_b, ln_g, ln_b,
           **run_kwargs):
    from concourse.bass_utils import run_bass_kernel_spmd

    nc = get_nc()
    in_maps = make_in_maps(q, k, v, s_valid_mask, idt,
                           qk_w, v_w, fc_w, fc_b, ln_g, ln_b)
    res = run_bass_kernel_spmd(nc, in_maps, core_ids=list(range(B)),
                               **run_kwargs)
    out = np.stack([res.results[b]["out"] for b in range(B)], axis=0)
    kernel.last_results = res
    return out.astype(np.float32)


# revision 26
# speedup vs baseline: 1.2066x; 1.0053x over previous
"""Trainium2 Bass kernel for nn_CrossAttention_17033840296537.

Full-input contract: kernel(**inputs) takes the unsharded tensors as in
reference.setup_inputs() and returns the full [8, 2048, 512] output.

Sharding: data-parallel over batch B=8 across the 8 NeuronCores (one
batch element per core). Weights are replicated.

Per-core design (bf16 matmul operands, f32 PSUM accumulation):
  prologue (everything SBUF-resident, no DRAM scratch):
    qk_w^T -> qkwT (bf16)
    q^T, k^T via PE transposes, projected to qhT/khT [512hd, 2048] bf16
    v cast to vt bf16 (lhsT for attn@V as stored)
    M_h = wv_h^T @ fc_w[:,h]^T  [512c, 512o] bf16 per head -- merges the
        v-projection and the output fc into ONE matmul stage downstream.
  main loop, j (512-query chunk) outer, h inner:
    scores^T[s,q] = khT[h] slices^T @ qhT[h]   (K=64, tile_position by
        head parity -- no duplication needed)
    pt = exp(scores*0.125 + mask_bias)  bf16, UNNORMALIZED
    t1[c,q]  += vt[s,c]^T pt           (K=128, PSUM accum over 16 s-blk)
    r[q]     += ones^T pt              (softmax denominator, same pass)
    fc partial fp[q,o] = sum_cb t1s[cb]^T M_h[cb]  (16 matmuls)
    facc[q,o] = fp * (1/r)[q] + facc   (ONE fused DVE op; 1/r arrives as
        a per-partition column via 4 tiny K=1 matmuls + reciprocal,
        entirely off the PE critical path)
  fc/facc/epilogue work of iteration i is emitted interleaved into
  iteration i+1's score/t1 stream so the PE never drains.
  epilogue per j: += idt, LayerNorm (Rsqrt on scalar, batched), -> out.
"""

import numpy as np

import concourse.bass as bass
import concourse.tile as tile
from concourse import mybir
from concourse.bass import ds
from concourse.masks import make_identity

F32 = mybir.dt.float32
BF = mybir.dt.bfloat16
I32 = mybir.dt.int32
AF = mybir.ActivationFunctionType
ALU = mybir.AluOpType

B = 8
NQ = NS = 2048
DIM = 512          # input channel dim (DIM_K == DIM_V == 512)
N_HEAD = 8
D_K = 64
D_V = 512
HD = N_HEAD * D_V  # 4096 concat dim
P = 128


def _emit(tc: tile.TileContext, io: dict):
    nc = tc.nc
    q, k, v, mask, idt = io["q"], io["k"], io["v"], io["mask"], io["idt"]
    qk_w, v_w, fc_w = io["qk_w"], io["v_w"], io["fc_w"]
    fc_b, ln_g, ln_b = io["fc_b"], io["ln_g"], io["ln_b"]
    out = io["out"]

    cpool_cm = tc.tile_pool(name="cpool", bufs=1)
    rpool_cm = tc.tile_pool(name="rpool", bufs=1)
    cpool = cpool_cm.__enter__()
    rpool = rpool_cm.__enter__()

    # ---- constants ----
    ident = cpool.tile([P, P], F32, name="ident")
    make_identity(nc, ident)
    ones_f = cpool.tile([P, 1], F32, name="ones_f")
    nc.vector.memset(ones_f, 1.0)
    ones_col = cpool.tile([P, 1], BF, name="ones_col")
    nc.vector.tensor_copy(out=ones_col, in_=ones_f)
    one11 = cpool.tile([1, 1], F32, name="one11")
    nc.vector.memset(one11, 1.0)
    eps_t = cpool.tile([P, 1], F32, name="eps_t")
    nc.vector.memset(eps_t, 1e-5)

    def bcast_row(name, src):  # [512] dram -> [128, 512] sbuf (rows identical)
        bc = cpool.tile([P, D_V], F32, name=name + "_bc")
        src_b = bass.AP(tensor=src.tensor, offset=src.offset,
                        ap=[[0, P]] + list(src.ap))
        nc.gpsimd.dma_start(out=bc, in_=src_b)
        return bc

    fcb_bc = bcast_row("fcb", fc_b)
    lng_bc = bcast_row("lng", ln_g)
    lnb_bc = bcast_row("lnb", ln_b)

    mask_i = cpool.tile([P, 16], I32, name="mask_i")
    nc.gpsimd.dma_start(out=mask_i, in_=mask.rearrange("(a p) -> p a", p=P))
    mask_b = cpool.tile([P, 16], F32, name="mask_b")
    nc.vector.tensor_copy(out=mask_b, in_=mask_i)  # int32 -> f32 cast
    nc.scalar.mul(mask_b, mask_b, -10000.0)

    # ---- residents ----
    vt = [rpool.tile([P, DIM], BF, name=f"vt{sb}") for sb in range(16)]
    qhT = [rpool.tile([P, NQ], BF, name=f"qhT{mb}") for mb in range(4)]
    khT = [rpool.tile([P, NS], BF, name=f"khT{mb}") for mb in range(4)]
    Msb = [[rpool.tile([P, D_V], BF, name=f"M{h}_{cb}") for cb in range(4)]
           for h in range(N_HEAD)]
    facc = [rpool.tile([P, D_V], F32, name=f"facc{i}") for i in range(16)]

    # ================= prologue =================
    # DMA: 512-row blocks as single [128, 2048] rearranged transfers.
    # sync queue:   qk_w, q, k           (feeds the projection pipeline)
    # gpsimd queue: fc_w/v_w per head, v (weight/value path), then casts
    def blk(src, r0, cols=512, c0=0):
        # 512 DRAM rows (cols c0:c0+cols) -> [128 p, 4*cols] view where
        # element (p, a*cols + c) = src[r0 + a*128 + p, c0 + c]
        rstr = src.ap[0][0]
        return bass.AP(tensor=src.tensor,
                       offset=src.offset + r0 * rstr + c0,
                       ap=[[rstr, P], [P * rstr, 4], [1, cols]])

    with (
        tc.tile_pool(name="pstage", bufs=1) as pstage,
        tc.tile_pool(name="ppsum", bufs=1, space="PSUM") as pp,
    ):
        # ---- all DMA triggers up front ----
        qkw_stg = pstage.tile([P, 2048], F32, name="qkw_stg", tag="qld",
                              bufs=2)
        nc.sync.dma_start(out=qkw_stg, in_=blk(qk_w, 0))
        qk_stg = []
        for src, sname in ((q, "q"), (k, "k")):
            for j2 in range(4):
                st = pstage.tile([P, 2048], F32, name=f"{sname}stg{j2}",
                                 tag="qld", bufs=2)
                nc.sync.dma_start(out=st, in_=blk(src, j2 * 512))
                qk_stg.append(st)
        wstg = []
        for h in range(N_HEAD):
            ft = pstage.tile([P, 2048], F32, name=f"fstg{h}", tag="wstg",
                             bufs=2)
            nc.gpsimd.dma_start(out=ft, in_=blk(fc_w, 0, c0=h * 512))
            vw_raw = pstage.tile([P, 2048], F32, name=f"vwstg{h}",
                                 tag="wstg", bufs=2)
            nc.gpsimd.dma_start(out=vw_raw, in_=blk(v_w, h * 512))
            wstg.append((ft, vw_raw))
        vstg = []
        for c4 in range(4):
            st = pstage.tile([P, 2048], F32, name=f"vstg{c4}", tag="vstg",
                             bufs=2)
            nc.gpsimd.dma_start(out=st, in_=blk(v, c4 * 512))
            vstg.append(st)

        # gpsimd: v_w casts per head (needed at M-build time)
        vwb = []
        for h in range(N_HEAD):
            vws = []
            for i in range(4):
                vb = pstage.tile([P, 512], BF, name=f"vwb{h}_{i}",
                                 tag="vwb", bufs=6)
                nc.gpsimd.tensor_copy(out=vb,
                                      in_=wstg[h][1][:, ds(i * 512, 512)])
                vws.append(vb)
            vwb.append(vws)

        # ---- qk_w^T -> qkwT bf16 ----
        qkwT = []
        for cb in range(4):
            tp = pp.tile([P, 512], F32, name=f"tpw{cb}", tag="tp", bufs=3)
            for rb in range(4):
                nc.tensor.transpose(tp[:, ds(rb * P, P)],
                                    qkw_stg[:, ds(rb * 512 + cb * P, P)],
                                    ident)
            qw = pstage.tile([P, 512], BF, name=f"qkwT{cb}", tag=f"qkwT{cb}")
            nc.vector.tensor_copy(out=qw, in_=tp)
            qkwT.append(qw)

        # ---- q, k: transpose + project -> qhT/khT bf16 (SBUF resident) ----
        for sidx, dstT in ((0, qhT), (1, khT)):
            for j2 in range(4):
                stg = qk_stg[sidx * 4 + j2]
                qTc = []
                for cb in range(4):
                    tp = pp.tile([P, 512], F32, name=f"tpq{sidx}{j2}_{cb}",
                                 tag="tp", bufs=3)
                    for qb in range(4):
                        nc.tensor.transpose(
                            tp[:, ds(qb * P, P)],
                            stg[:, ds(qb * 512 + cb * P, P)], ident)
                    qc = pstage.tile([P, 512], BF, name=f"qTc{j2}_{cb}",
                                     tag="qTc", bufs=4)
                    nc.vector.tensor_copy(out=qc, in_=tp)
                    qTc.append(qc)
                for mb in range(4):
                    pr = pp.tile([P, 512], F32, name=f"pr{j2}_{mb}",
                                 tag="pr", bufs=2)
                    for cb in range(4):
                        nc.tensor.matmul(pr, lhsT=qkwT[cb][:, ds(mb * P, P)],
                                         rhs=qTc[cb],
                                         start=(cb == 0), stop=(cb == 3))
                    nc.scalar.copy(out=dstT[mb][:, ds(j2 * 512, 512)], in_=pr)

        # ---- per-head merged projection M_h = wv_h^T @ fc_w[:,h]^T ----
        for h in range(N_HEAD):
            fstg = wstg[h][0]
            fwT = []
            for db in range(4):
                tp = pp.tile([P, 512], F32, name=f"tpf{h}_{db}",
                             tag="tp", bufs=3)
                for rb in range(4):
                    nc.tensor.transpose(
                        tp[:, ds(rb * P, P)],
                        fstg[:, ds(rb * 512 + db * P, P)], ident)
                fw = pstage.tile([P, 512], BF, name=f"fwT{h}_{db}",
                                 tag="fwT", bufs=4)
                nc.vector.tensor_copy(out=fw, in_=tp)
                fwT.append(fw)
            # two v casts per head on DVE -- spreads them so vt is ready
            # just before the main loop consumes it
            for sb in (2 * h, 2 * h + 1):
                nc.vector.tensor_copy(
                    out=vt[sb], in_=vstg[sb // 4][:, ds((sb % 4) * 512, 512)])
            for cb in range(4):
                pr = pp.tile([P, 512], F32, name=f"prM{h}_{cb}",
                             tag="pr", bufs=2)
                for i in range(4):
                    nc.tensor.matmul(pr, lhsT=vwb[h][i][:, ds(cb * P, P)],
                                     rhs=fwT[i],
                                     start=(i == 0), stop=(i == 3))
                nc.scalar.copy(out=Msb[h][cb], in_=pr)

    # ================= main =================
    with (
        tc.tile_pool(name="ms", bufs=1) as ms,
        tc.tile_pool(name="mp", bufs=1, space="PSUM") as mp,
    ):
        iters = [(j, h) for j in range(4) for h in range(8)]
        idt_tiles = {}   # j -> [4 tiles]
        prev = None      # dict carrying previous iteration's state
        ep_pending = []  # j values whose epilogue is ready to emit

        def emit_idt_loads(j):
            it = ms.tile([P, 2048], F32, name=f"idt{j}", tag="idt", bufs=1)
            nc.sync.dma_start(out=it, in_=blk(idt, j * 512))
            idt_tiles[j] = it

        def emit_rcol(pv, idx):
            # previous iteration's softmax sums [1,512] -> per-partition
            # column [128,4] + reciprocal; rides the "fp" PSUM bank.
            rcolt = mp.tile([P, 512], F32, name=f"rcol{idx}", tag="fp",
                            bufs=1)
            for qb in range(4):
                nc.tensor.matmul(rcolt[:, ds(qb, 1)],
                                 lhsT=pv["rs"][0:1, ds(qb * P, P)],
                                 rhs=one11, start=True, stop=True)
            rinv = ms.tile([P, 4], F32, name=f"rinv{idx}", tag="rinv", bufs=2)
            nc.vector.reciprocal(rinv, rcolt[:, 0:4])
            pv["rinv"] = rinv

        def emit_fc_group(pv, qb, idx):
            fpt = mp.tile([P, 512], F32, name=f"fp{idx}_{qb}", tag="fp",
                          bufs=1)
            for cb in range(4):
                nc.tensor.matmul(fpt,
                                 lhsT=pv["t1s"][cb][:, ds(qb * P, P)],
                                 rhs=Msb[pv["h"]][cb],
                                 start=(cb == 0), stop=(cb == 3))
            i16 = pv["j"] * 4 + qb
            in1 = fcb_bc if pv["h"] == 0 else facc[i16]
            nc.vector.scalar_tensor_tensor(out=facc[i16], in0=fpt,
                                           scalar=pv["rinv"][:, ds(qb, 1)],
                                           in1=in1,
                                           op0=ALU.mult, op1=ALU.add)
            if pv["h"] == 7 and qb == 3:
                ep_pending.append(pv["j"])

        def emit_epilogue(j):
            # residual + LayerNorm, in place on the facc tiles
            xts, mvs = [], []
            for qb in range(4):
                i16 = j * 4 + qb
                xt = facc[i16]
                nc.vector.tensor_add(xt, xt,
                                     idt_tiles[j][:, ds(qb * 512, 512)])
                st = ms.tile([P, 6], F32, name=f"st{i16}", tag="st", bufs=4)
                nc.vector.bn_stats(out=st, in_=xt)
                mv = ms.tile([P, 2], F32, name=f"mv{i16}", tag="mv", bufs=4)
                nc.vector.bn_aggr(out=mv, in_=st)
                xts.append(xt)
                mvs.append(mv)
            sds = []
            for qb in range(4):  # batched so the scalar engine swaps its
                i16 = j * 4 + qb  # activation table Exp->Sqrt only once
                sd = ms.tile([P, 1], F32, name=f"sd{i16}", tag="sd", bufs=4)
                nc.scalar.activation(sd, mvs[qb][:, 1:2], AF.Sqrt,
                                     bias=eps_t)
                sds.append(sd)
            rstds = []
            for qb in range(4):
                i16 = j * 4 + qb
                rstd = ms.tile([P, 1], F32, name=f"rstd{i16}", tag="rstd",
                               bufs=4)
                nc.vector.reciprocal(rstd, sds[qb])
                rstds.append(rstd)
            for qb in range(4):
                i16 = j * 4 + qb
                xt = xts[qb]
                nc.vector.tensor_scalar(out=xt, in0=xt,
                                        scalar1=mvs[qb][:, 0:1],
                                        scalar2=rstds[qb],
                                        op0=ALU.subtract, op1=ALU.mult)
                nc.vector.tensor_mul(xt, xt, lng_bc)
                nc.vector.tensor_add(xt, xt, lnb_bc)
                nc.sync.dma_start(out=out[ds(i16 * P, P), :], in_=xt)

        for idx, (j, h) in enumerate(iters):
            par = h % 2
            tnum = h // 2
            po = par * D_K
            t1 = mp.tile([P, 4 * 512], F32, name=f"t1_{idx}", tag="t1",
                         bufs=1)
            r2a = mp.tile([1, 512], F32, name=f"r2a_{idx}", tag="r2a",
                          bufs=1)

            def emit_pair(si):
                tiles = []
                for sb in (2 * si, 2 * si + 1):
                    sct = mp.tile([P, 512], F32, name=f"sc{idx}_{sb}",
                                  tag="sc", bufs=2)
                    nc.tensor.matmul(sct,
                                     lhsT=khT[tnum][po:po + D_K,
                                                    ds(sb * P, P)],
                                     rhs=qhT[tnum][po:po + D_K,
                                                   ds(j * 512, 512)],
                                     start=True, stop=True,
                                     tile_position=(po, 0))
                    tiles.append(sct)
                return tiles

            def emit_half(sb, sct):
                ptt = ms.tile([P, 512], BF, name=f"pt{idx}_{sb}", tag="pt",
                              bufs=4)
                nc.scalar.activation(ptt, sct, AF.Exp,
                                     bias=mask_b[:, ds(sb, 1)], scale=0.125)
                for cb in range(4):
                    nc.tensor.matmul(t1[:, ds(cb * 512, 512)],
                                     lhsT=vt[sb][:, ds(cb * P, P)],
                                     rhs=ptt,
                                     start=(sb == 0), stop=(sb == 15))
                nc.tensor.matmul(r2a, lhsT=ones_col, rhs=ptt,
                                 start=(sb == 0), stop=(sb == 15))

            pair = emit_pair(0)
            if prev is not None:
                emit_rcol(prev, idx)
            for si in range(8):
                emit_half(2 * si, pair[0])
                if si < 7:
                    nxt = emit_pair(si + 1)
                if prev is not None and 1 <= si <= 4:
                    emit_fc_group(prev, si - 1, idx)
                if si == 5:
                    if h == 5:
                        emit_idt_loads(j)
                    if ep_pending and h >= 1:
                        emit_epilogue(ep_pending.pop(0))
                emit_half(2 * si + 1, pair[1])
                if si < 7:
                    pair = nxt

            # end of iteration: softmax sums out, t1 -> SBUF bf16 split
            # across scalar+vector (gpsimd cannot read PSUM) to minimize
            # the t1 PSUM free latency
            rs = ms.tile([1, 512], F32, name=f"rs{idx}", tag="rs", bufs=2)
            nc.vector.tensor_copy(out=rs, in_=r2a)
            t1s = [ms.tile([P, 512], BF, name=f"t1s{idx}_{cb}", tag="t1s",
                           bufs=5) for cb in range(4)]
            nc.scalar.copy(out=t1s[0], in_=t1[:, ds(0, 512)])
            nc.scalar.copy(out=t1s[1], in_=t1[:, ds(512, 512)])
            nc.vector.tensor_copy(out=t1s[2], in_=t1[:, ds(1024, 512)])
            nc.vector.tensor_copy(out=t1s[3], in_=t1[:, ds(1536, 512)])
            prev = {"j": j, "h": h, "t1s": t1s, "rs": rs, "rinv": None}

        # drain the last iteration + epilogue for j=3
        emit_rcol(prev, 32)
        for qb in range(4):
            emit_fc_group(prev, qb, 32)
        while ep_pending:
            emit_epilogue(ep_pending.pop(0))

    rpool_cm.__exit__(None, None, None)
    cpool_cm.__exit__(None, None, None)


def build_nc():
    from concourse import bacc
    nc = bacc.Bacc("TRN2", target_bir_lowering=False, debug=False)
    io = {}
    io["q"] = nc.dram_tensor("q", [NQ, DIM], F32, kind="ExternalInput").ap()
    io["k"] = nc.dram_tensor("k", [NS, DIM], F32, kind="ExternalInput").ap()
    io["v"] = nc.dram_tensor("v", [NS, DIM], F32, kind="ExternalInput").ap()
    io["mask"] = nc.dram_tensor("mask", [NS], I32, kind="ExternalInput").ap()
    io["idt"] = nc.dram_tensor("idt", [NQ, D_V], F32, kind="ExternalInput").ap()
    io["qk_w"] = nc.dram_tensor("qk_w", [512, DIM], F32, kind="ExternalInput").ap()
    io["v_w"] = nc.dram_tensor("v_w", [HD, DIM], F32, kind="ExternalInput").ap()
    io["fc_w"] = nc.dram_tensor("fc_w", [D_V, HD], F32, kind="ExternalInput").ap()
    io["fc_b"] = nc.dram_tensor("fc_b", [D_V], F32, kind="ExternalInput").ap()
    io["ln_g"] = nc.dram_tensor("ln_g", [D_V], F32, kind="ExternalInput").ap()
    io["ln_b"] = nc.dram_tensor("ln_b", [D_V], F32, kind="ExternalInput").ap()
    io["out"] = nc.dram_tensor("out", [NQ, D_V], F32, kind="ExternalOutput").ap()

    with tile.TileContext(nc) as tc:
        _emit(tc, io)
    nc.compile()
    return nc


_NC = None


def get_nc():
    global _NC
    if _NC is None:
        _NC = build_nc()
    return _NC


def make_in_maps(q, k, v, s_valid_mask, idt, qk_w, v_w, fc_w, fc_b, ln_g, ln_b):
    in_maps = []
    for b in range(B):
        in_maps.append({
            "q": np.ascontiguousarray(q[b], dtype=np.float32),
            "k": np.ascontiguousarray(k[b], dtype=np.float32),
            "v": np.ascontiguousarray(v[b], dtype=np.float32),
            "mask": np.ascontiguousarray(s_valid_mask[b], dtype=np.int32),
            "idt": np.ascontiguousarray(idt[b], dtype=np.float32),
            "qk_w": np.ascontiguousarray(qk_w, dtype=np.float32),
            "v_w": np.ascontiguousarray(v_w, dtype=np.float32),
            "fc_w": np.ascontiguousarray(fc_w, dtype=np.float32),
            "fc_b": np.ascontiguousarray(fc_b, dtype=np.float32),
            "ln_g": np.ascontiguousarray(ln_g, dtype=np.float32),
            "ln_b": np.ascontiguousarray(ln_b, dtype=np.float32),
        })
    return in_maps


def kernel(q, k, v, s_valid_mask, idt, qk_w, v_w, fc_w, fc_b, ln_g, ln_b,
           **run_kwargs):
    from concourse.bass_utils import run_bass_kernel_spmd

    nc = get_nc()
    in_maps = make_in_maps(q, k, v, s_valid_mask, idt,
                           qk_w, v_w, fc_w, fc_b, ln_g, ln_b)
    res = run_bass_kernel_spmd(nc, in_maps, core_ids=list(range(B)),
                               **run_kwargs)
    out = np.stack([res.results[b]["out"] for b in range(B)], axis=0)
    kernel.last_results = res
    return out.astype(np.float32)


# revision 29
# speedup vs baseline: 1.2387x; 1.0267x over previous
"""Trainium2 Bass kernel for nn_CrossAttention_17033840296537.

Full-input contract: kernel(**inputs) takes the unsharded tensors as in
reference.setup_inputs() and returns the full [8, 2048, 512] output.

Sharding: data-parallel over batch B=8 across the 8 NeuronCores (one
batch element per core). Weights are replicated.

Per-core design (bf16 matmul operands, f32 PSUM accumulation):
  prologue (everything SBUF-resident, no DRAM scratch):
    qk_w^T -> qkwT (bf16)
    q^T, k^T via PE transposes, projected to qhT/khT [512hd, 2048] bf16
    v cast to vt bf16 (lhsT for attn@V as stored)
    M_h = wv_h^T @ fc_w[:,h]^T  [512c, 512o] bf16 per head -- merges the
        v-projection and the output fc into ONE matmul stage downstream.
  main loop, j (512-query chunk) outer, h inner:
    scores^T[s,q] = khT[h] slices^T @ qhT[h]   (K=64, tile_position by
        head parity -- no duplication needed)
    pt = exp(scores*0.125 + mask_bias)  bf16, UNNORMALIZED
    t1[c,q]  += vt[s,c]^T pt           (K=128, PSUM accum over 16 s-blk)
    r[q]     += ones^T pt              (softmax denominator, same pass)
    fc partial fp[q,o] = sum_cb t1s[cb]^T M_h[cb]  (16 matmuls)
    facc[q,o] = fp * (1/r)[q] + facc   (ONE fused DVE op; 1/r arrives as
        a per-partition column via 4 tiny K=1 matmuls + reciprocal,
        entirely off the PE critical path)
  fc/facc/epilogue work of iteration i is emitted interleaved into
  iteration i+1's score/t1 stream so the PE never drains.
  epilogue per j: += idt, LayerNorm (Rsqrt on scalar, batched), -> out.
"""

import numpy as np

import concourse.bass as bass
import concourse.tile as tile
from concourse import mybir
from concourse.bass import ds
from concourse.masks import make_identity

F32 = mybir.dt.float32
BF = mybir.dt.bfloat16
I32 = mybir.dt.int32
AF = mybir.ActivationFunctionType
ALU = mybir.AluOpType

B = 8
NQ = NS = 2048
DIM = 512          # input channel dim (DIM_K == DIM_V == 512)
N_HEAD = 8
D_K = 64
D_V = 512
HD = N_HEAD * D_V  # 4096 concat dim
P = 128


def _emit(tc: tile.TileContext, io: dict):
    nc = tc.nc
    q, k, v, mask, idt = io["q"], io["k"], io["v"], io["mask"], io["idt"]
    qk_w, v_w, fc_w = io["qk_w"], io["v_w"], io["fc_w"]
    fc_b, ln_g, ln_b = io["fc_b"], io["ln_g"], io["ln_b"]
    out = io["out"]

    cpool_cm = tc.tile_pool(name="cpool", bufs=1)
    rpool_cm = tc.tile_pool(name="rpool", bufs=1)
    cpool = cpool_cm.__enter__()
    rpool = rpool_cm.__enter__()

    # ---- constants ----
    ident = cpool.tile([P, P], F32, name="ident")
    make_identity(nc, ident)
    ones_f = cpool.tile([P, 1], F32, name="ones_f")
    nc.vector.memset(ones_f, 1.0)
    ones_col = cpool.tile([P, 1], BF, name="ones_col")
    nc.vector.tensor_copy(out=ones_col, in_=ones_f)
    one11 = cpool.tile([1, 1], F32, name="one11")
    nc.vector.memset(one11, 1.0)
    eps_t = cpool.tile([P, 1], F32, name="eps_t")
    nc.vector.memset(eps_t, 1e-5)

    def bcast_row(name, src):  # [512] dram -> [128, 512] sbuf (rows identical)
        bc = cpool.tile([P, D_V], F32, name=name + "_bc")
        src_b = bass.AP(tensor=src.tensor, offset=src.offset,
                        ap=[[0, P]] + list(src.ap))
        nc.gpsimd.dma_start(out=bc, in_=src_b)
        return bc

    fcb_bc = bcast_row("fcb", fc_b)
    lng_bc = bcast_row("lng", ln_g)
    lnb_bc = bcast_row("lnb", ln_b)

    mask_i = cpool.tile([P, 16], I32, name="mask_i")
    nc.gpsimd.dma_start(out=mask_i, in_=mask.rearrange("(a p) -> p a", p=P))
    mask_b = cpool.tile([P, 16], F32, name="mask_b")
    nc.vector.tensor_copy(out=mask_b, in_=mask_i)  # int32 -> f32 cast
    nc.scalar.mul(mask_b, mask_b, -10000.0)

    # ---- residents ----
    vt = [rpool.tile([P, DIM], BF, name=f"vt{sb}") for sb in range(16)]
    qhT = [rpool.tile([P, NQ], BF, name=f"qhT{mb}") for mb in range(4)]
    khT = [rpool.tile([P, NS], BF, name=f"khT{mb}") for mb in range(4)]
    Msb = [[rpool.tile([P, D_V], BF, name=f"M{h}_{cb}") for cb in range(4)]
           for h in range(N_HEAD)]
    facc = [rpool.tile([P, D_V], F32, name=f"facc{i}") for i in range(16)]

    # ================= prologue =================
    # DMA: 512-row blocks as single [128, 2048] rearranged transfers.
    # sync queue:   qk_w, q, k           (feeds the projection pipeline)
    # gpsimd queue: fc_w/v_w per head, v (weight/value path), then casts
    def blk(src, r0, cols=512, c0=0):
        # 512 DRAM rows (cols c0:c0+cols) -> [128 p, 4*cols] view where
        # element (p, a*cols + c) = src[r0 + a*128 + p, c0 + c]
        rstr = src.ap[0][0]
        return bass.AP(tensor=src.tensor,
                       offset=src.offset + r0 * rstr + c0,
                       ap=[[rstr, P], [P * rstr, 4], [1, cols]])

    with (
        tc.tile_pool(name="pstage", bufs=1) as pstage,
        tc.tile_pool(name="ppsum", bufs=1, space="PSUM") as pp,
    ):
        # ---- all DMA triggers up front ----
        qkw_stg = pstage.tile([P, 2048], F32, name="qkw_stg", tag="qld",
                              bufs=2)
        nc.sync.dma_start(out=qkw_stg, in_=blk(qk_w, 0))
        qk_stg = []
        for src, sname in ((q, "q"), (k, "k")):
            for j2 in range(4):
                st = pstage.tile([P, 2048], F32, name=f"{sname}stg{j2}",
                                 tag="qld", bufs=2)
                nc.sync.dma_start(out=st, in_=blk(src, j2 * 512))
                qk_stg.append(st)
        wstg = []
        for h in range(N_HEAD):
            ft = pstage.tile([P, 2048], F32, name=f"fstg{h}", tag="wstg",
                             bufs=2)
            nc.gpsimd.dma_start(out=ft, in_=blk(fc_w, 0, c0=h * 512))
            vw_raw = pstage.tile([P, 2048], F32, name=f"vwstg{h}",
                                 tag="wstg", bufs=2)
            nc.gpsimd.dma_start(out=vw_raw, in_=blk(v_w, h * 512))
            wstg.append((ft, vw_raw))
        vstg = []
        for c4 in range(4):
            st = pstage.tile([P, 2048], F32, name=f"vstg{c4}", tag="vstg",
                             bufs=2)
            nc.gpsimd.dma_start(out=st, in_=blk(v, c4 * 512))
            vstg.append(st)

        # v_w casts per head, split gpsimd/scalar so neither queue exceeds
        # the PE's ~5.1us per-head M-build pace
        vwb = []
        for h in range(N_HEAD):
            vws = []
            for i in range(4):
                vb = pstage.tile([P, 512], BF, name=f"vwb{h}_{i}",
                                 tag="vwb", bufs=6)
                eng = nc.gpsimd if i < 2 else nc.scalar
                if eng is nc.gpsimd:
                    nc.gpsimd.tensor_copy(out=vb,
                                          in_=wstg[h][1][:, ds(i * 512, 512)])
                else:
                    nc.scalar.copy(out=vb,
                                   in_=wstg[h][1][:, ds(i * 512, 512)])
                vws.append(vb)
            vwb.append(vws)

        # ---- qk_w^T -> qkwT bf16 ----
        qkwT = []
        for cb in range(4):
            tp = pp.tile([P, 512], F32, name=f"tpw{cb}", tag="tp", bufs=3)
            for rb in range(4):
                nc.tensor.transpose(tp[:, ds(rb * P, P)],
                                    qkw_stg[:, ds(rb * 512 + cb * P, P)],
                                    ident)
            qw = pstage.tile([P, 512], BF, name=f"qkwT{cb}", tag=f"qkwT{cb}")
            nc.vector.tensor_copy(out=qw, in_=tp)
            qkwT.append(qw)

        # ---- q, k: transpose + project -> qhT/khT bf16 (SBUF resident) ----
        for sidx, dstT in ((0, qhT), (1, khT)):
            for j2 in range(4):
                stg = qk_stg[sidx * 4 + j2]
                qTc = []
                for cb in range(4):
                    tp = pp.tile([P, 512], F32, name=f"tpq{sidx}{j2}_{cb}",
                                 tag="tp", bufs=3)
                    for qb in range(4):
                        nc.tensor.transpose(
                            tp[:, ds(qb * P, P)],
                            stg[:, ds(qb * 512 + cb * P, P)], ident)
                    qc = pstage.tile([P, 512], BF, name=f"qTc{j2}_{cb}",
                                     tag="qTc", bufs=4)
                    nc.vector.tensor_copy(out=qc, in_=tp)
                    qTc.append(qc)
                # cb-outer so each matmul wave depends on only one qTc copy
                prs = [pp.tile([P, 512], F32, name=f"pr{j2}_{mb}",
                               tag="pr", bufs=4) for mb in range(4)]
                for cb in range(4):
                    for mb in range(4):
                        nc.tensor.matmul(prs[mb],
                                         lhsT=qkwT[cb][:, ds(mb * P, P)],
                                         rhs=qTc[cb],
                                         start=(cb == 0), stop=(cb == 3))
                for mb in range(4):
                    nc.scalar.copy(out=dstT[mb][:, ds(j2 * 512, 512)],
                                   in_=prs[mb])

        # ---- per-head merged projection M_h = wv_h^T @ fc_w[:,h]^T ----
        for h in range(N_HEAD):
            fstg = wstg[h][0]
            fwT = []
            for db in range(4):
                tp = pp.tile([P, 512], F32, name=f"tpf{h}_{db}",
                             tag="tp", bufs=3)
                for rb in range(4):
                    nc.tensor.transpose(
                        tp[:, ds(rb * P, P)],
                        fstg[:, ds(rb * 512 + db * P, P)], ident)
                fw = pstage.tile([P, 512], BF, name=f"fwT{h}_{db}",
                                 tag="fwT", bufs=4)
                nc.vector.tensor_copy(out=fw, in_=tp)
                fwT.append(fw)
            # two v casts per head on DVE -- spreads them so vt is ready
            # just before the main loop consumes it
            for sb in (2 * h, 2 * h + 1):
                nc.vector.tensor_copy(
                    out=vt[sb], in_=vstg[sb // 4][:, ds((sb % 4) * 512, 512)])
            # i-outer so each matmul wave depends on only one fwT copy
            prs = [pp.tile([P, 512], F32, name=f"prM{h}_{cb}",
                           tag="pr", bufs=4) for cb in range(4)]
            for i in range(4):
                for cb in range(4):
                    nc.tensor.matmul(prs[cb],
                                     lhsT=vwb[h][i][:, ds(cb * P, P)],
                                     rhs=fwT[i],
                                     start=(i == 0), stop=(i == 3))
            for cb in range(4):
                nc.scalar.copy(out=Msb[h][cb], in_=prs[cb])

    # ================= main =================
    with (
        tc.tile_pool(name="ms", bufs=1) as ms,
        tc.tile_pool(name="mp", bufs=1, space="PSUM") as mp,
    ):
        iters = [(j, h) for j in range(4) for h in range(8)]
        idt_tiles = {}   # j -> [4 tiles]
        prev = None      # dict carrying previous iteration's state
        ep_pending = []  # j values whose epilogue is ready to emit

        def emit_idt_loads(j):
            it = ms.tile([P, 2048], F32, name=f"idt{j}", tag="idt", bufs=1)
            nc.sync.dma_start(out=it, in_=blk(idt, j * 512))
            idt_tiles[j] = it

        def emit_rcol(pv, idx):
            # previous iteration's softmax sums [1,512] -> per-partition
            # column [128,4] + reciprocal; rides the "fp" PSUM bank.
            rcolt = mp.tile([P, 512], F32, name=f"rcol{idx}", tag="fp",
                            bufs=1)
            for qb in range(4):
                nc.tensor.matmul(rcolt[:, ds(qb, 1)],
                                 lhsT=pv["rs"][0:1, ds(qb * P, P)],
                                 rhs=one11, start=True, stop=True)
            rinv = ms.tile([P, 4], F32, name=f"rinv{idx}", tag="rinv", bufs=2)
            nc.vector.reciprocal(rinv, rcolt[:, 0:4])
            pv["rinv"] = rinv

        def emit_fc_group(pv, qb, idx):
            fpt = mp.tile([P, 512], F32, name=f"fp{idx}_{qb}", tag="fp",
                          bufs=1)
            for cb in range(4):
                nc.tensor.matmul(fpt,
                                 lhsT=pv["t1s"][cb][:, ds(qb * P, P)],
                                 rhs=Msb[pv["h"]][cb],
                                 start=(cb == 0), stop=(cb == 3))
            i16 = pv["j"] * 4 + qb
            in1 = fcb_bc if pv["h"] == 0 else facc[i16]
            nc.vector.scalar_tensor_tensor(out=facc[i16], in0=fpt,
                                           scalar=pv["rinv"][:, ds(qb, 1)],
                                           in1=in1,
                                           op0=ALU.mult, op1=ALU.add)
            if pv["h"] == 7 and qb == 3:
                ep_pending.append(pv["j"])

        def emit_epilogue(j):
            # residual + LayerNorm, in place on the facc tiles
            xts, mvs = [], []
            for qb in range(4):
                i16 = j * 4 + qb
                xt = facc[i16]
                nc.vector.tensor_add(xt, xt,
                                     idt_tiles[j][:, ds(qb * 512, 512)])
                st = ms.tile([P, 6], F32, name=f"st{i16}", tag="st", bufs=4)
                nc.vector.bn_stats(out=st, in_=xt)
                mv = ms.tile([P, 2], F32, name=f"mv{i16}", tag="mv", bufs=4)
                nc.vector.bn_aggr(out=mv, in_=st)
                xts.append(xt)
                mvs.append(mv)
            sds = []
            for qb in range(4):  # batched so the scalar engine swaps its
                i16 = j * 4 + qb  # activation table Exp->Sqrt only once
                sd = ms.tile([P, 1], F32, name=f"sd{i16}", tag="sd", bufs=4)
                nc.scalar.activation(sd, mvs[qb][:, 1:2], AF.Sqrt,
                                     bias=eps_t)
                sds.append(sd)
            rstds = []
            for qb in range(4):
                i16 = j * 4 + qb
                rstd = ms.tile([P, 1], F32, name=f"rstd{i16}", tag="rstd",
                               bufs=4)
                nc.vector.reciprocal(rstd, sds[qb])
                rstds.append(rstd)
            for qb in range(4):
                i16 = j * 4 + qb
                xt = xts[qb]
                nc.vector.tensor_scalar(out=xt, in0=xt,
                                        scalar1=mvs[qb][:, 0:1],
                                        scalar2=rstds[qb],
                                        op0=ALU.subtract, op1=ALU.mult)
                nc.vector.tensor_mul(xt, xt, lng_bc)
                nc.vector.tensor_add(xt, xt, lnb_bc)
                nc.sync.dma_start(out=out[ds(i16 * P, P), :], in_=xt)

        for idx, (j, h) in enumerate(iters):
            par = h % 2
            tnum = h // 2
            po = par * D_K
            t1 = mp.tile([P, 4 * 512], F32, name=f"t1_{idx}", tag="t1",
                         bufs=1)
            r2a = mp.tile([1, 512], F32, name=f"r2a_{idx}", tag="r2a",
                          bufs=1)

            def emit_pair(si):
                tiles = []
                for sb in (2 * si, 2 * si + 1):
                    sct = mp.tile([P, 512], F32, name=f"sc{idx}_{sb}",
                                  tag="sc", bufs=2)
                    nc.tensor.matmul(sct,
                                     lhsT=khT[tnum][po:po + D_K,
                                                    ds(sb * P, P)],
                                     rhs=qhT[tnum][po:po + D_K,
                                                   ds(j * 512, 512)],
                                     start=True, stop=True,
                                     tile_position=(po, 0))
                    tiles.append(sct)
                return tiles

            def emit_half(sb, sct):
                ptt = ms.tile([P, 512], BF, name=f"pt{idx}_{sb}", tag="pt",
                              bufs=4)
                nc.scalar.activation(ptt, sct, AF.Exp,
                                     bias=mask_b[:, ds(sb, 1)], scale=0.125)
                for cb in range(4):
                    nc.tensor.matmul(t1[:, ds(cb * 512, 512)],
                                     lhsT=vt[sb][:, ds(cb * P, P)],
                                     rhs=ptt,
                                     start=(sb == 0), stop=(sb == 15))
                nc.tensor.matmul(r2a, lhsT=ones_col, rhs=ptt,
                                 start=(sb == 0), stop=(sb == 15))

            pair = emit_pair(0)
            if prev is not None:
                emit_rcol(prev, idx)
            for si in range(8):
                emit_half(2 * si, pair[0])
                if si < 7:
                    nxt = emit_pair(si + 1)
                if prev is not None and 1 <= si <= 4:
                    emit_fc_group(prev, si - 1, idx)
                if si == 5:
                    if h == 5:
                        emit_idt_loads(j)
                    if ep_pending and h >= 1:
                        emit_epilogue(ep_pending.pop(0))
                emit_half(2 * si + 1, pair[1])
                if si < 7:
                    pair = nxt

            # end of iteration: softmax sums out, t1 -> SBUF bf16 split
            # across scalar+vector (gpsimd cannot read PSUM) to minimize
            # the t1 PSUM free latency
            rs = ms.tile([1, 512], F32, name=f"rs{idx}", tag="rs", bufs=2)
            nc.vector.tensor_copy(out=rs, in_=r2a)
            t1s = [ms.tile([P, 512], BF, name=f"t1s{idx}_{cb}", tag="t1s",
                           bufs=5) for cb in range(4)]
            nc.scalar.copy(out=t1s[0], in_=t1[:, ds(0, 512)])
            nc.scalar.copy(out=t1s[1], in_=t1[:, ds(512, 512)])
            nc.vector.tensor_copy(out=t1s[2], in_=t1[:, ds(1024, 512)])
            nc.vector.tensor_copy(out=t1s[3], in_=t1[:, ds(1536, 512)])
            prev = {"j": j, "h": h, "t1s": t1s, "rs": rs, "rinv": None}

        # drain the last iteration + epilogue for j=3
        emit_rcol(prev, 32)
        for qb in range(4):
            emit_fc_group(prev, qb, 32)
        while ep_pending:
            emit_epilogue(ep_pending.pop(0))

    rpool_cm.__exit__(None, None, None)
    cpool_cm.__exit__(None, None, None)


def build_nc():
    from concourse import bacc
    nc = bacc.Bacc("TRN2", target_bir_lowering=False, debug=False)
    io = {}
    io["q"] = nc.dram_tensor("q", [NQ, DIM], F32, kind="ExternalInput").ap()
    io["k"] = nc.dram_tensor("k", [NS, DIM], F32, kind="ExternalInput").ap()
    io["v"] = nc.dram_tensor("v", [NS, DIM], F32, kind="ExternalInput").ap()
    io["mask"] = nc.dram_tensor("mask", [NS], I32, kind="ExternalInput").ap()
    io["idt"] = nc.dram_tensor("idt", [NQ, D_V], F32, kind="ExternalInput").ap()
    io["qk_w"] = nc.dram_tensor("qk_w", [512, DIM], F32, kind="ExternalInput").ap()
    io["v_w"] = nc.dram_tensor("v_w", [HD, DIM], F32, kind="ExternalInput").ap()
    io["fc_w"] = nc.dram_tensor("fc_w", [D_V, HD], F32, kind="ExternalInput").ap()
    io["fc_b"] = nc.dram_tensor("fc_b", [D_V], F32, kind="ExternalInput").ap()
    io["ln_g"] = nc.dram_tensor("ln_g", [D_V], F32, kind="ExternalInput").ap()
    io["ln_b"] = nc.dram_tensor("ln_b", [D_V], F32, kind="ExternalInput").ap()
    io["out"] = nc.dram_tensor("out", [NQ, D_V], F32, kind="ExternalOutput").ap()

    with tile.TileContext(nc) as tc:
        _emit(tc, io)
    nc.compile()
    return nc


_NC = None


def get_nc():
    global _NC
    if _NC is None:
        _NC = build_nc()
    return _NC


def make_in_maps(q, k, v, s_valid_mask, idt, qk_w, v_w, fc_w, fc_b, ln_g, ln_b):
    in_maps = []
    for b in range(B):
        in_maps.append({
            "q": np.ascontiguousarray(q[b], dtype=np.float32),
            "k": np.ascontiguousarray(k[b], dtype=np.float32),
            "v": np.ascontiguousarray(v[b], dtype=np.float32),
            "mask": np.ascontiguousarray(s_valid_mask[b], dtype=np.int32),
            "idt": np.ascontiguousarray(idt[b], dtype=np.float32),
            "qk_w": np.ascontiguousarray(qk_w, dtype=np.float32),
            "v_w": np.ascontiguousarray(v_w, dtype=np.float32),
            "fc_w": np.ascontiguousarray(fc_w, dtype=np.float32),
            "fc_b": np.ascontiguousarray(fc_b, dtype=np.float32),
            "ln_g": np.ascontiguousarray(ln_g, dtype=np.float32),
            "ln_b": np.ascontiguousarray(ln_b, dtype=np.float32),
        })
    return in_maps


def kernel(q, k, v, s_valid_mask, idt, qk_w, v_w, fc_w, fc_b, ln_g, ln_b,
           **run_kwargs):
    from concourse.bass_utils import run_bass_kernel_spmd

    nc = get_nc()
    in_maps = make_in_maps(q, k, v, s_valid_mask, idt,
                           qk_w, v_w, fc_w, fc_b, ln_g, ln_b)
    res = run_bass_kernel_spmd(nc, in_maps, core_ids=list(range(B)),
                               **run_kwargs)
    out = np.stack([res.results[b]["out"] for b in range(B)], axis=0)
    kernel.last_results = res
    return out.astype(np.float32)


# revision 33
# speedup vs baseline: 1.2741x; 1.0286x over previous
"""Trainium2 Bass kernel for nn_CrossAttention_17033840296537.

Full-input contract: kernel(**inputs) takes the unsharded tensors as in
reference.setup_inputs() and returns the full [8, 2048, 512] output.

Sharding: data-parallel over batch B=8 across the 8 NeuronCores (one
batch element per core). Weights are replicated.

Per-core design (bf16 matmul operands, f32 PSUM accumulation):
  prologue (everything SBUF-resident, no DRAM scratch):
    qk_w^T -> qkwT (bf16)
    q^T, k^T via PE transposes, projected to qhT/khT [512hd, 2048] bf16
    v cast to vt bf16 (lhsT for attn@V as stored)
    M_h = wv_h^T @ fc_w[:,h]^T  [512c, 512o] bf16 per head -- merges the
        v-projection and the output fc into ONE matmul stage downstream.
  main loop, j (512-query chunk) outer, h inner:
    scores^T[s,q] = khT[h] slices^T @ qhT[h]   (K=64, tile_position by
        head parity -- no duplication needed)
    pt = exp(scores*0.125 + mask_bias)  bf16, UNNORMALIZED
    t1[c,q]  += vt[s,c]^T pt           (K=128, PSUM accum over 16 s-blk)
    r[q]     += ones^T pt              (softmax denominator, same pass)
    fc partial fp[q,o] = sum_cb t1s[cb]^T M_h[cb]  (16 matmuls)
    facc[q,o] = fp * (1/r)[q] + facc   (ONE fused DVE op; 1/r arrives as
        a per-partition column via 4 tiny K=1 matmuls + reciprocal,
        entirely off the PE critical path)
  fc/facc/epilogue work of iteration i is emitted interleaved into
  iteration i+1's score/t1 stream so the PE never drains.
  epilogue per j: += idt, LayerNorm (Rsqrt on scalar, batched), -> out.
"""

import numpy as np

import concourse.bass as bass
import concourse.tile as tile
from concourse import mybir
from concourse.bass import ds
from concourse.masks import make_identity

F32 = mybir.dt.float32
BF = mybir.dt.bfloat16
I32 = mybir.dt.int32
AF = mybir.ActivationFunctionType
ALU = mybir.AluOpType

B = 8
NQ = NS = 2048
DIM = 512          # input channel dim (DIM_K == DIM_V == 512)
N_HEAD = 8
D_K = 64
D_V = 512
HD = N_HEAD * D_V  # 4096 concat dim
P = 128


def _emit(tc: tile.TileContext, io: dict):
    nc = tc.nc
    q, k, v, mask, idt = io["q"], io["k"], io["v"], io["mask"], io["idt"]
    qk_w, v_w, fc_w = io["qk_w"], io["v_w"], io["fc_w"]
    fc_b, ln_g, ln_b = io["fc_b"], io["ln_g"], io["ln_b"]
    out = io["out"]

    cpool_cm = tc.tile_pool(name="cpool", bufs=1)
    rpool_cm = tc.tile_pool(name="rpool", bufs=1)
    cpool = cpool_cm.__enter__()
    rpool = rpool_cm.__enter__()

    # ---- constants ----
    ident = cpool.tile([P, P], F32, name="ident")
    make_identity(nc, ident)
    ones_f = cpool.tile([P, 1], F32, name="ones_f")
    nc.vector.memset(ones_f, 1.0)
    ones_col = cpool.tile([P, 1], BF, name="ones_col")
    nc.vector.tensor_copy(out=ones_col, in_=ones_f)
    one11 = cpool.tile([1, 1], F32, name="one11")
    nc.vector.memset(one11, 1.0)
    eps_t = cpool.tile([P, 1], F32, name="eps_t")
    nc.vector.memset(eps_t, 1e-5)

    def bcast_row(name, src):  # [512] dram -> [128, 512] sbuf (rows identical)
        bc = cpool.tile([P, D_V], F32, name=name + "_bc")
        src_b = bass.AP(tensor=src.tensor, offset=src.offset,
                        ap=[[0, P]] + list(src.ap))
        nc.gpsimd.dma_start(out=bc, in_=src_b)
        return bc

    fcb_bc = bcast_row("fcb", fc_b)
    lng_bc = bcast_row("lng", ln_g)
    lnb_bc = bcast_row("lnb", ln_b)

    mask_i = cpool.tile([P, 16], I32, name="mask_i")
    nc.gpsimd.dma_start(out=mask_i, in_=mask.rearrange("(a p) -> p a", p=P))
    mask_b = cpool.tile([P, 16], F32, name="mask_b")
    nc.vector.tensor_copy(out=mask_b, in_=mask_i)  # int32 -> f32 cast
    nc.scalar.mul(mask_b, mask_b, -10000.0)

    # ---- residents ----
    vt = [rpool.tile([P, DIM], BF, name=f"vt{sb}") for sb in range(16)]
    qhT = [rpool.tile([P, NQ], BF, name=f"qhT{mb}") for mb in range(4)]
    khT = [rpool.tile([P, NS], BF, name=f"khT{mb}") for mb in range(4)]
    Msb = [[rpool.tile([P, D_V], BF, name=f"M{h}_{cb}") for cb in range(4)]
           for h in range(N_HEAD)]
    facc = [rpool.tile([P, D_V], F32, name=f"facc{i}") for i in range(16)]

    # ================= prologue =================
    # DMA: 512-row blocks as single [128, 2048] rearranged transfers.
    # sync queue:   qk_w, q, k           (feeds the projection pipeline)
    # gpsimd queue: fc_w/v_w per head, v (weight/value path), then casts
    def blk(src, r0, cols=512, c0=0):
        # 512 DRAM rows (cols c0:c0+cols) -> [128 p, 4*cols] view where
        # element (p, a*cols + c) = src[r0 + a*128 + p, c0 + c]
        rstr = src.ap[0][0]
        return bass.AP(tensor=src.tensor,
                       offset=src.offset + r0 * rstr + c0,
                       ap=[[rstr, P], [P * rstr, 4], [1, cols]])

    with (
        tc.tile_pool(name="pstage", bufs=1) as pstage,
        tc.tile_pool(name="ppsum", bufs=1, space="PSUM") as pp,
    ):
        # ---- all DMA triggers up front ----
        # sync queue in priority order (q,k feed the PE first, then the
        # per-head weights) so transfers don't compete for HBM bandwidth;
        # v rides the gpsimd queue concurrently (small, needed mid-phase).
        qkw_stg = pstage.tile([P, 2048], F32, name="qkw_stg", tag="qld",
                              bufs=3)
        nc.sync.dma_start(out=qkw_stg, in_=blk(qk_w, 0))
        qk_stg = []
        for src, sname in ((q, "q"), (k, "k")):
            for j2 in range(4):
                st = pstage.tile([P, 2048], F32, name=f"{sname}stg{j2}",
                                 tag="qld", bufs=3)
                nc.sync.dma_start(out=st, in_=blk(src, j2 * 512))
                qk_stg.append(st)
        vstg = []
        for c4 in range(4):
            st = pstage.tile([P, 2048], F32, name=f"vstg{c4}", tag="vstg",
                             bufs=2)
            nc.gpsimd.dma_start(out=st, in_=blk(v, c4 * 512))
            vstg.append(st)
        wstg = []
        for h in range(N_HEAD):
            ft = pstage.tile([P, 2048], F32, name=f"fstg{h}", tag="wstg",
                             bufs=3)
            nc.sync.dma_start(out=ft, in_=blk(fc_w, 0, c0=h * 512))
            vw_raw = pstage.tile([P, 2048], F32, name=f"vwstg{h}",
                                 tag="wstg", bufs=3)
            nc.sync.dma_start(out=vw_raw, in_=blk(v_w, h * 512))
            wstg.append((ft, vw_raw))

        # v_w casts per head, split gpsimd/scalar so neither queue exceeds
        # the PE's ~5.1us per-head M-build pace
        vwb = []
        for h in range(N_HEAD):
            vws = []
            for i in range(4):
                vb = pstage.tile([P, 512], BF, name=f"vwb{h}_{i}",
                                 tag="vwb", bufs=5)
                eng = nc.gpsimd if i < 2 else nc.scalar
                if eng is nc.gpsimd:
                    nc.gpsimd.tensor_copy(out=vb,
                                          in_=wstg[h][1][:, ds(i * 512, 512)])
                else:
                    nc.scalar.copy(out=vb,
                                   in_=wstg[h][1][:, ds(i * 512, 512)])
                vws.append(vb)
            vwb.append(vws)

        # ---- qk_w^T -> qkwT bf16 ----
        qkwT = []
        for cb in range(4):
            tp = pp.tile([P, 512], F32, name=f"tpw{cb}", tag="tp", bufs=3)
            for rb in range(4):
                nc.tensor.transpose(tp[:, ds(rb * P, P)],
                                    qkw_stg[:, ds(rb * 512 + cb * P, P)],
                                    ident)
            qw = pstage.tile([P, 512], BF, name=f"qkwT{cb}", tag=f"qkwT{cb}")
            nc.vector.tensor_copy(out=qw, in_=tp)
            qkwT.append(qw)

        # ---- q, k: transpose + project -> qhT/khT bf16 (SBUF resident) ----
        # transposes run one chunk ahead of the projection waves so the
        # DVE qTc copies are always hidden behind PE work
        def emit_qkT(ci):
            stg = qk_stg[ci]
            qTc = []
            for cb in range(4):
                tp = pp.tile([P, 512], F32, name=f"tpq{ci}_{cb}",
                             tag="tp", bufs=3)
                for qb in range(4):
                    nc.tensor.transpose(
                        tp[:, ds(qb * P, P)],
                        stg[:, ds(qb * 512 + cb * P, P)], ident)
                qc = pstage.tile([P, 512], BF, name=f"qTc{ci}_{cb}",
                                 tag="qTc", bufs=8)
                nc.vector.tensor_copy(out=qc, in_=tp)
                qTc.append(qc)
            return qTc

        def emit_proj(ci, qTc):
            dstT = qhT if ci < 4 else khT
            j2 = ci % 4
            # cb-outer so each matmul wave depends on only one qTc copy
            prs = [pp.tile([P, 512], F32, name=f"pr{ci}_{mb}",
                           tag="pr", bufs=4) for mb in range(4)]
            for cb in range(4):
                for mb in range(4):
                    nc.tensor.matmul(prs[mb],
                                     lhsT=qkwT[cb][:, ds(mb * P, P)],
                                     rhs=qTc[cb],
                                     start=(cb == 0), stop=(cb == 3))
            for mb in range(4):
                nc.scalar.copy(out=dstT[mb][:, ds(j2 * 512, 512)],
                               in_=prs[mb])

        qTc_cur = emit_qkT(0)
        for ci in range(8):
            qTc_nxt = emit_qkT(ci + 1) if ci < 7 else None
            emit_proj(ci, qTc_cur)
            qTc_cur = qTc_nxt

        # ---- per-head merged projection M_h = wv_h^T @ fc_w[:,h]^T ----
        # fwT transposes run one head ahead of the M matmul waves
        def emit_fwT(h):
            fstg = wstg[h][0]
            fwT = []
            for db in range(4):
                tp = pp.tile([P, 512], F32, name=f"tpf{h}_{db}",
                             tag="tp", bufs=3)
                for rb in range(4):
                    nc.tensor.transpose(
                        tp[:, ds(rb * P, P)],
                        fstg[:, ds(rb * 512 + db * P, P)], ident)
                fw = pstage.tile([P, 512], BF, name=f"fwT{h}_{db}",
                                 tag="fwT", bufs=8)
                nc.vector.tensor_copy(out=fw, in_=tp)
                fwT.append(fw)
            return fwT

        def emit_M(h, fwT):
            # two v casts per head on DVE -- spreads them so vt is ready
            # just before the main loop consumes it
            for sb in (2 * h, 2 * h + 1):
                nc.vector.tensor_copy(
                    out=vt[sb], in_=vstg[sb // 4][:, ds((sb % 4) * 512, 512)])
            # i-outer so each matmul wave depends on only one fwT copy
            prs = [pp.tile([P, 512], F32, name=f"prM{h}_{cb}",
                           tag="pr", bufs=4) for cb in range(4)]
            for i in range(4):
                for cb in range(4):
                    nc.tensor.matmul(prs[cb],
                                     lhsT=vwb[h][i][:, ds(cb * P, P)],
                                     rhs=fwT[i],
                                     start=(i == 0), stop=(i == 3))
            for cb in range(4):
                nc.scalar.copy(out=Msb[h][cb], in_=prs[cb])

        fwT_cur = emit_fwT(0)
        for h in range(N_HEAD):
            fwT_nxt = emit_fwT(h + 1) if h < 7 else None
            emit_M(h, fwT_cur)
            fwT_cur = fwT_nxt

    # ================= main =================
    with (
        tc.tile_pool(name="ms", bufs=1) as ms,
        tc.tile_pool(name="mp", bufs=1, space="PSUM") as mp,
    ):
        iters = [(j, h) for j in range(4) for h in range(8)]
        idt_tiles = {}   # j -> [4 tiles]
        prev = None      # dict carrying previous iteration's state
        ep_pending = []  # j values whose epilogue is ready to emit

        def emit_idt_loads(j):
            it = ms.tile([P, 2048], F32, name=f"idt{j}", tag="idt", bufs=1)
            nc.sync.dma_start(out=it, in_=blk(idt, j * 512))
            idt_tiles[j] = it

        def emit_rcol(pv, idx):
            # previous iteration's softmax sums [1,512] -> per-partition
            # column [128,4] + reciprocal; rides the "fp" PSUM bank.
            rcolt = mp.tile([P, 512], F32, name=f"rcol{idx}", tag="fp",
                            bufs=1)
            for qb in range(4):
                nc.tensor.matmul(rcolt[:, ds(qb, 1)],
                                 lhsT=pv["rs"][0:1, ds(qb * P, P)],
                                 rhs=one11, start=True, stop=True)
            rinv = ms.tile([P, 4], F32, name=f"rinv{idx}", tag="rinv", bufs=2)
            nc.vector.reciprocal(rinv, rcolt[:, 0:4])
            pv["rinv"] = rinv

        def emit_fc_group(pv, qb, idx):
            fpt = mp.tile([P, 512], F32, name=f"fp{idx}_{qb}", tag="fp",
                          bufs=1)
            for cb in range(4):
                nc.tensor.matmul(fpt,
                                 lhsT=pv["t1s"][cb][:, ds(qb * P, P)],
                                 rhs=Msb[pv["h"]][cb],
                                 start=(cb == 0), stop=(cb == 3))
            i16 = pv["j"] * 4 + qb
            in1 = fcb_bc if pv["h"] == 0 else facc[i16]
            nc.vector.scalar_tensor_tensor(out=facc[i16], in0=fpt,
                                           scalar=pv["rinv"][:, ds(qb, 1)],
                                           in1=in1,
                                           op0=ALU.mult, op1=ALU.add)
            if pv["h"] == 7 and qb == 3:
                ep_pending.append(pv["j"])

        def emit_epilogue(j):
            # residual + LayerNorm, in place on the facc tiles
            xts, mvs = [], []
            for qb in range(4):
                i16 = j * 4 + qb
                xt = facc[i16]
                nc.vector.tensor_add(xt, xt,
                                     idt_tiles[j][:, ds(qb * 512, 512)])
                st = ms.tile([P, 6], F32, name=f"st{i16}", tag="st", bufs=4)
                nc.vector.bn_stats(out=st, in_=xt)
                mv = ms.tile([P, 2], F32, name=f"mv{i16}", tag="mv", bufs=4)
                nc.vector.bn_aggr(out=mv, in_=st)
                xts.append(xt)
                mvs.append(mv)
            sds = []
            for qb in range(4):  # batched so the scalar engine swaps its
                i16 = j * 4 + qb  # activation table Exp->Sqrt only once
                sd = ms.tile([P, 1], F32, name=f"sd{i16}", tag="sd", bufs=4)
                nc.scalar.activation(sd, mvs[qb][:, 1:2], AF.Sqrt,
                                     bias=eps_t)
                sds.append(sd)
            rstds = []
            for qb in range(4):
                i16 = j * 4 + qb
                rstd = ms.tile([P, 1], F32, name=f"rstd{i16}", tag="rstd",
                               bufs=4)
                nc.vector.reciprocal(rstd, sds[qb])
                rstds.append(rstd)
            for qb in range(4):
                i16 = j * 4 + qb
                xt = xts[qb]
                nc.vector.tensor_scalar(out=xt, in0=xt,
                                        scalar1=mvs[qb][:, 0:1],
                                        scalar2=rstds[qb],
                                        op0=ALU.subtract, op1=ALU.mult)
                nc.vector.tensor_mul(xt, xt, lng_bc)
                nc.vector.tensor_add(xt, xt, lnb_bc)
                nc.sync.dma_start(out=out[ds(i16 * P, P), :], in_=xt)

        for idx, (j, h) in enumerate(iters):
            par = h % 2
            tnum = h // 2
            po = par * D_K
            t1 = mp.tile([P, 4 * 512], F32, name=f"t1_{idx}", tag="t1",
                         bufs=1)
            r2a = mp.tile([1, 512], F32, name=f"r2a_{idx}", tag="r2a",
                          bufs=1)

            def emit_pair(si):
                tiles = []
                for sb in (2 * si, 2 * si + 1):
                    sct = mp.tile([P, 512], F32, name=f"sc{idx}_{sb}",
                                  tag="sc", bufs=2)
                    nc.tensor.matmul(sct,
                                     lhsT=khT[tnum][po:po + D_K,
                                                    ds(sb * P, P)],
                                     rhs=qhT[tnum][po:po + D_K,
                                                   ds(j * 512, 512)],
                                     start=True, stop=True,
                                     tile_position=(po, 0))
                    tiles.append(sct)
                return tiles

            def emit_half(sb, sct):
                ptt = ms.tile([P, 512], BF, name=f"pt{idx}_{sb}", tag="pt",
                              bufs=4)
                nc.scalar.activation(ptt, sct, AF.Exp,
                                     bias=mask_b[:, ds(sb, 1)], scale=0.125)
                for cb in range(4):
                    nc.tensor.matmul(t1[:, ds(cb * 512, 512)],
                                     lhsT=vt[sb][:, ds(cb * P, P)],
                                     rhs=ptt,
                                     start=(sb == 0), stop=(sb == 15))
                nc.tensor.matmul(r2a, lhsT=ones_col, rhs=ptt,
                                 start=(sb == 0), stop=(sb == 15))

            pair = emit_pair(0)
            if prev is not None:
                emit_rcol(prev, idx)
            for si in range(8):
                emit_half(2 * si, pair[0])
                if si < 7:
                    nxt = emit_pair(si + 1)
                if prev is not None and 1 <= si <= 4:
                    emit_fc_group(prev, si - 1, idx)
                if si == 5:
                    if h == 5:
                        emit_idt_loads(j)
                    if ep_pending and h >= 1:
                        emit_epilogue(ep_pending.pop(0))
                emit_half(2 * si + 1, pair[1])
                if si < 7:
                    pair = nxt

            # end of iteration: softmax sums out, t1 -> SBUF bf16 split
            # across scalar+vector (gpsimd cannot read PSUM) to minimize
            # the t1 PSUM free latency
            rs = ms.tile([1, 512], F32, name=f"rs{idx}", tag="rs", bufs=2)
            nc.vector.tensor_copy(out=rs, in_=r2a)
            t1s = [ms.tile([P, 512], BF, name=f"t1s{idx}_{cb}", tag="t1s",
                           bufs=5) for cb in range(4)]
            nc.scalar.copy(out=t1s[0], in_=t1[:, ds(0, 512)])
            nc.scalar.copy(out=t1s[1], in_=t1[:, ds(512, 512)])
            nc.vector.tensor_copy(out=t1s[2], in_=t1[:, ds(1024, 512)])
            nc.vector.tensor_copy(out=t1s[3], in_=t1[:, ds(1536, 512)])
            prev = {"j": j, "h": h, "t1s": t1s, "rs": rs, "rinv": None}

        # drain the last iteration + epilogue for j=3
        emit_rcol(prev, 32)
        for qb in range(4):
            emit_fc_group(prev, qb, 32)
        while ep_pending:
            emit_epilogue(ep_pending.pop(0))

    rpool_cm.__exit__(None, None, None)
    cpool_cm.__exit__(None, None, None)


def build_nc():
    from concourse import bacc
    nc = bacc.Bacc("TRN2", target_bir_lowering=False, debug=False)
    io = {}
    io["q"] = nc.dram_tensor("q", [NQ, DIM], F32, kind="ExternalInput").ap()
    io["k"] = nc.dram_tensor("k", [NS, DIM], F32, kind="ExternalInput").ap()
    io["v"] = nc.dram_tensor("v", [NS, DIM], F32, kind="ExternalInput").ap()
    io["mask"] = nc.dram_tensor("mask", [NS], I32, kind="ExternalInput").ap()
    io["idt"] = nc.dram_tensor("idt", [NQ, D_V], F32, kind="ExternalInput").ap()
    io["qk_w"] = nc.dram_tensor("qk_w", [512, DIM], F32, kind="ExternalInput").ap()
    io["v_w"] = nc.dram_tensor("v_w", [HD, DIM], F32, kind="ExternalInput").ap()
    io["fc_w"] = nc.dram_tensor("fc_w", [D_V, HD], F32, kind="ExternalInput").ap()
    io["fc_b"] = nc.dram_tensor("fc_b", [D_V], F32, kind="ExternalInput").ap()
    io["ln_g"] = nc.dram_tensor("ln_g", [D_V], F32, kind="ExternalInput").ap()
    io["ln_b"] = nc.dram_tensor("ln_b", [D_V], F32, kind="ExternalInput").ap()
    io["out"] = nc.dram_tensor("out", [NQ, D_V], F32, kind="ExternalOutput").ap()

    with tile.TileContext(nc) as tc:
        _emit(tc, io)
    nc.compile()
    return nc


_NC = None


def get_nc():
    global _NC
    if _NC is None:
        _NC = build_nc()
    return _NC


def make_in_maps(q, k, v, s_valid_mask, idt, qk_w, v_w, fc_w, fc_b, ln_g, ln_b):
    in_maps = []
    for b in range(B):
        in_maps.append({
            "q": np.ascontiguousarray(q[b], dtype=np.float32),
            "k": np.ascontiguousarray(k[b], dtype=np.float32),
            "v": np.ascontiguousarray(v[b], dtype=np.float32),
            "mask": np.ascontiguousarray(s_valid_mask[b], dtype=np.int32),
            "idt": np.ascontiguousarray(idt[b], dtype=np.float32),
            "qk_w": np.ascontiguousarray(qk_w, dtype=np.float32),
            "v_w": np.ascontiguousarray(v_w, dtype=np.float32),
            "fc_w": np.ascontiguousarray(fc_w, dtype=np.float32),
            "fc_b": np.ascontiguousarray(fc_b, dtype=np.float32),
            "ln_g": np.ascontiguousarray(ln_g, dtype=np.float32),
            "ln_b": np.ascontiguousarray(ln_b, dtype=np.float32),
        })
    return in_maps


def kernel(q, k, v, s_valid_mask, idt, qk_w, v_w, fc_w, fc_b, ln_g, ln_b,
           **run_kwargs):
    from concourse.bass_utils import run_bass_kernel_spmd

    nc = get_nc()
    in_maps = make_in_maps(q, k, v, s_valid_mask, idt,
                           qk_w, v_w, fc_w, fc_b, ln_g, ln_b)
    res = run_bass_kernel_spmd(nc, in_maps, core_ids=list(range(B)),
                               **run_kwargs)
    out = np.stack([res.results[b]["out"] for b in range(B)], axis=0)
    kernel.last_results = res
    return out.astype(np.float32)
